# revision 4
# baseline (speedup 1.0000x reference)
"""Distributed Bass kernel for nn_ASAGE (GraphSAGE over a two-tower user/item
graph) on 8 TRN2 NeuronCores.

Strategy:
  Phase 1 (node embeddings, node-sharded): each core computes x = proj(feats)
  and v0 = x @ (W_v0/20) for its 1/8 slice of users+items. Embedding-bag
  lookups (feature bags, text tokens) run as big int16 `dma_gather`s from
  DRAM tables; dense features are host-pre-transposed so they DMA straight
  into feature-major matmul operands.
  AllGather of [x||v0] (one [19456,128] f32 tile per core -> [155648,128]).
  Phase 2 (data-parallel over seeds): per-core 256 seeds; v0 rows for
  neighbor aggregation gathered via indirect DMA (int32 indices); two
  GraphSAGE layers as feature-major matmuls.

Host-side work is restricted to layout transformations: slicing/sharding,
integer index remapping/reordering, weight concat + folding of the constant
bag-mean scales (1/10, 1/8, 1/20) into the corresponding weight rows.
"""
import os
import sys
import types

sys.path.insert(0, "/opt/trn_rl_repo")

import numpy as np


def _ensure_axon_hooks():
    """Provide antenv.axon_hooks + register the NTFF profile hook (the agent
    image's antenv lacks it). Needed only for trace=True timing runs."""
    try:
        import antenv
    except ImportError:
        return
    if "antenv.axon_hooks" in sys.modules:
        return
    mod = types.ModuleType("antenv.axon_hooks")
    mod._HOOK = None

    def set_axon_ntff_profile_hook(hook, _mod=mod):
        _mod._HOOK = hook

    def get_axon_ntff_profile_hook(_mod=mod):
        return _mod._HOOK

    mod.set_axon_ntff_profile_hook = set_axon_ntff_profile_hook
    mod.get_axon_ntff_profile_hook = get_axon_ntff_profile_hook
    sys.modules["antenv.axon_hooks"] = mod
    antenv.axon_hooks = mod
    try:
        from trn_agent_boot.trn_boot import _ntff_profile_via_ctypes

        mod.set_axon_ntff_profile_hook(
            _ntff_profile_via_ctypes("/opt/axon/libaxon_pjrt.so")
        )
    except Exception:
        pass


_ensure_axon_hooks()

import concourse.bass as bass
import concourse.bacc as bacc
import concourse.tile as tile
import concourse.mybir as mybir
from concourse.bass_utils import run_bass_kernel_spmd
from concourse.masks import make_identity

# ---- problem constants (hardcoded per contract) ----
N_USER = 100000
M_ITEM = 50000
D = 64
WD = 32
B = 2048
S = 20
VOCAB = 20000
NCORES = 8

U_PC = N_USER // NCORES          # 12500 real users per core
I_PC = M_ITEM // NCORES          # 6250 real items per core
U_PAD = 12800                    # 25 tiles of 512
I_PAD = 6656                     # 13 tiles of 512
NT_U = U_PAD // 512
NT_I = I_PAD // 512
NODES_PC = U_PAD + I_PAD         # 19456
XV_ROWS = NODES_PC * NCORES      # 155648
B_PC = B // NCORES               # 256
PAIRS = B_PC * S                 # 5120
NCH2 = 8                         # neigh2 chunks
PAIRS_CH = PAIRS // NCH2         # 640 = 5 * 128

F32 = mybir.dt.float32
I32 = mybir.dt.int32
I16 = mybir.dt.int16

LAST_RESULT = None  # BassKernelResults of the most recent run (for test.py)

_CACHED = None  # (nc, static_in_map) — compile once per process


def _remap_nodes(idx):
    """Map reference node ids (users 0..1e5, items 1e5..1.5e5) to AG row ids."""
    idx = np.asarray(idx, dtype=np.int64)
    user = idx < N_USER
    out = np.empty_like(idx)
    u = idx[user]
    out[user] = (u // U_PC) * NODES_PC + (u % U_PC)
    m = idx[~user] - N_USER
    out[~user] = (m // I_PC) * NODES_PC + U_PAD + (m % I_PC)
    return out.astype(np.int32)


def _wrap_idx(flat):
    """dma_gather int16 index layout: [128, n/16], idx for element i at
    partition i%16 (replicated to all 8 16-partition groups), free slot i//16."""
    flat = np.asarray(flat, dtype=np.int16)
    n = flat.shape[-1]
    lead = flat.shape[:-1]
    w = flat.reshape(*lead, n // 16, 16)
    w = np.swapaxes(w, -1, -2)  # [..., 16, n//16]
    return np.tile(w, (1,) * len(lead) + (8, 1)).astype(np.int16)


def _bag_sum(nc, pool, g, nsub, bag, e, name):
    """Sum over the `bag` axis of g viewed as [128, nsub, bag, e].
    Returns a contiguous [128, nsub, e] tile. bag in (8, 10, 20)."""
    v = g.rearrange("p (j b) e -> p j b e", b=bag)
    if bag == 8:
        a1 = pool.tile([128, nsub, 4, e], F32, tag=f"{name}a1", name=f"{name}a1")
        nc.vector.tensor_tensor(out=a1[:], in0=v[:, :, 0:4, :], in1=v[:, :, 4:8, :],
                                op=mybir.AluOpType.add)
        a2 = pool.tile([128, nsub, 2, e], F32, tag=f"{name}a2", name=f"{name}a2")
        nc.vector.tensor_tensor(out=a2[:], in0=a1[:, :, 0:2, :], in1=a1[:, :, 2:4, :],
                                op=mybir.AluOpType.add)
        a3 = pool.tile([128, nsub, e], F32, tag=f"{name}a3", name=f"{name}a3")
        nc.vector.tensor_tensor(out=a3[:], in0=a2[:, :, 0, :], in1=a2[:, :, 1, :],
                                op=mybir.AluOpType.add)
        return a3
    if bag == 10:
        s1 = pool.tile([128, nsub, 5, e], F32, tag=f"{name}s1", name=f"{name}s1")
        nc.vector.tensor_tensor(out=s1[:], in0=v[:, :, 0:5, :], in1=v[:, :, 5:10, :],
                                op=mybir.AluOpType.add)
        s2 = pool.tile([128, nsub, 2, e], F32, tag=f"{name}s2", name=f"{name}s2")
        nc.vector.tensor_tensor(out=s2[:], in0=s1[:, :, 0:2, :], in1=s1[:, :, 2:4, :],
                                op=mybir.AluOpType.add)
        s3 = pool.tile([128, nsub, e], F32, tag=f"{name}s3", name=f"{name}s3")
        nc.vector.tensor_tensor(out=s3[:], in0=s2[:, :, 0, :], in1=s2[:, :, 1, :],
                                op=mybir.AluOpType.add)
        s4 = pool.tile([128, nsub, e], F32, tag=f"{name}s4", name=f"{name}s4")
        nc.vector.tensor_tensor(out=s4[:], in0=s3[:], in1=s1[:, :, 4, :],
                                op=mybir.AluOpType.add)
        return s4
    assert bag == 20
    a = pool.tile([128, nsub, 10, e], F32, tag=f"{name}a", name=f"{name}a")
    nc.vector.tensor_tensor(out=a[:], in0=v[:, :, 0:10, :], in1=v[:, :, 10:20, :],
                            op=mybir.AluOpType.add)
    b5 = pool.tile([128, nsub, 5, e], F32, tag=f"{name}b", name=f"{name}b")
    nc.vector.tensor_tensor(out=b5[:], in0=a[:, :, 0:5, :], in1=a[:, :, 5:10, :],
                            op=mybir.AluOpType.add)
    c2 = pool.tile([128, nsub, 2, e], F32, tag=f"{name}c", name=f"{name}c")
    nc.vector.tensor_tensor(out=c2[:], in0=b5[:, :, 0:2, :], in1=b5[:, :, 2:4, :],
                            op=mybir.AluOpType.add)
    d = pool.tile([128, nsub, e], F32, tag=f"{name}d", name=f"{name}d")
    nc.vector.tensor_tensor(out=d[:], in0=c2[:, :, 0, :], in1=c2[:, :, 1, :],
                            op=mybir.AluOpType.add)
    ee = pool.tile([128, nsub, e], F32, tag=f"{name}e", name=f"{name}e")
    nc.vector.tensor_tensor(out=ee[:], in0=d[:], in1=b5[:, :, 4, :],
                            op=mybir.AluOpType.add)
    return ee


def _build_program():
    nc = bacc.Bacc("TRN2", target_bir_lowering=False, debug=False,
                   num_devices=NCORES, num_swdge_queues=4)
    dt = nc.dram_tensor

    # dense (host pre-transposed, padded) per-core inputs
    uidT = dt("uidT", [64, U_PAD], F32, kind="ExternalInput")
    uw300T = dt("uw300T", [300, U_PAD], F32, kind="ExternalInput")
    unumT = dt("unumT", [10, U_PAD], F32, kind="ExternalInput")
    iidT = dt("iidT", [64, I_PAD], F32, kind="ExternalInput")
    iw300T = dt("iw300T", [300, I_PAD], F32, kind="ExternalInput")
    is768T = dt("is768T", [768, I_PAD], F32, kind="ExternalInput")
    inumT = dt("inumT", [10, I_PAD], F32, kind="ExternalInput")

    # gather tables (replicated)
    ufe = dt("ufe", [30000, 64], F32, kind="ExternalInput")
    ife = dt("ife", [30000, 64], F32, kind="ExternalInput")
    wordp = dt("wordp", [VOCAB, 64], F32, kind="ExternalInput")  # padded 32->64

    # gather indices
    ufidx = dt("ufidx", [NT_U, 128, 320], I16, kind="ExternalInput")
    utidx = dt("utidx", [NT_U, 128, 768], I16, kind="ExternalInput")
    ifidx = dt("ifidx", [NT_I, 128, 320], I16, kind="ExternalInput")
    itidx = dt("itidx", [NT_I, 128, 768], I16, kind="ExternalInput")
    nb2idx = dt("nb2idx", [NCH2, 100, 128], I32, kind="ExternalInput")
    x1idx = dt("x1idx", [NCH2, 5, 128], I32, kind="ExternalInput")
    nb1sidx = dt("nb1sidx", [40, 128], I32, kind="ExternalInput")
    seedidx = dt("seedidx", [2, 128], I32, kind="ExternalInput")

    # weights (replicated; host-packed, scales folded)
    W_user = dt("W_user", [534, 64], F32, kind="ExternalInput")
    b_user = dt("b_user", [64], F32, kind="ExternalInput")
    W_item = dt("W_item", [1302, 64], F32, kind="ExternalInput")
    b_item = dt("b_item", [64], F32, kind="ExternalInput")
    Wv0 = dt("Wv0", [64, 64], F32, kind="ExternalInput")     # /20 folded
    bv0 = dt("bv0", [64], F32, kind="ExternalInput")
    Wv1 = dt("Wv1", [64, 64], F32, kind="ExternalInput")     # /20 folded
    bv1 = dt("bv1", [64], F32, kind="ExternalInput")
    Ww0t = dt("Ww0t", [64, 64], F32, kind="ExternalInput")
    Ww0b = dt("Ww0b", [64, 64], F32, kind="ExternalInput")
    bw0 = dt("bw0", [64], F32, kind="ExternalInput")
    Ww1t = dt("Ww1t", [64, 64], F32, kind="ExternalInput")
    Ww1b = dt("Ww1b", [64, 64], F32, kind="ExternalInput")
    bw1 = dt("bw1", [64], F32, kind="ExternalInput")

    out = dt("out", [B_PC, 64], F32, kind="ExternalOutput")

    # user/item segment row-ranges in W_user/W_item
    useg = [("id", 0, 64), ("feat", 64, 128), ("text", 128, 224),
            ("w3a", 224, 352), ("w3b", 352, 480), ("w3c", 480, 524),
            ("num", 524, 534)]
    iseg = [("id", 0, 64), ("feat", 64, 128), ("text", 128, 224),
            ("w3a", 224, 352), ("w3b", 352, 480), ("w3c", 480, 524),
            ("s7a", 524, 652), ("s7b", 652, 780), ("s7c", 780, 908),
            ("s7d", 908, 1036), ("s7e", 1036, 1164), ("s7f", 1164, 1292),
            ("num", 1292, 1302)]

    with tile.TileContext(nc) as tc:
        with tc.tile_pool(name="const", bufs=1) as constp, \
             tc.tile_pool(name="dram", bufs=1, space="DRAM") as dram:
            ident = constp.tile([128, 128], F32)
            make_identity(nc, ident[:])

            def load_w(t, rows=64):
                w = constp.tile([rows, 64], F32, name=f"w_{t.name}", uniquify=True)
                nc.sync.dma_start(out=w[:], in_=t[:])
                return w

            def load_b(t):
                b_ = constp.tile([64, 1], F32, name=f"b_{t.name}", uniquify=True)
                nc.sync.dma_start(out=b_[:], in_=t[:, None])
                return b_

            wu = {}
            for nm, r0, r1 in useg:
                w = constp.tile([r1 - r0, 64], F32, name=f"wu_{nm}")
                nc.sync.dma_start(out=w[:], in_=W_user[r0:r1, :])
                wu[nm] = w
            wi = {}
            for nm, r0, r1 in iseg:
                w = constp.tile([r1 - r0, 64], F32, name=f"wi_{nm}")
                nc.sync.dma_start(out=w[:], in_=W_item[r0:r1, :])
                wi[nm] = w
            bu_t = load_b(b_user)
            bi_t = load_b(b_item)
            wv0_t = load_w(Wv0)
            bv0_t = load_b(bv0)
            wv1_t = load_w(Wv1)
            bv1_t = load_b(bv1)
            ww0t_t = load_w(Ww0t)
            ww0b_t = load_w(Ww0b)
            bw0_t = load_b(bw0)
            ww1t_t = load_w(Ww1t)
            ww1b_t = load_w(Ww1b)
            bw1_t = load_b(bw1)

            xv_slice = dram.tile([NODES_PC, 128], F32)
            xv_full = dram.tile([XV_ROWS, 128], F32, addr_space="Shared")

            # ---------------- phase 1 ----------------
            with tc.tile_pool(name="p1", bufs=2) as p1, \
                 tc.tile_pool(name="p1r", bufs=2) as p1r, \
                 tc.tile_pool(name="ps_acc", bufs=2, space="PSUM") as ps_acc, \
                 tc.tile_pool(name="ps_tr", bufs=2, space="PSUM") as ps_tr, \
                 tc.tile_pool(name="ps_v0", bufs=1, space="PSUM") as ps_v0:

                def node_tile(kind, t):
                    if kind == "u":
                        segs, wseg, bseg = useg, wu, bu_t
                        fidx, tidx, ftab = ufidx, utidx, ufe
                        idT, w300T, numT, s768T = uidT, uw300T, unumT, None
                        row0 = t * 512
                    else:
                        segs, wseg, bseg = iseg, wi, bi_t
                        fidx, tidx, ftab = ifidx, itidx, ife
                        idT, w300T, numT, s768T = iidT, iw300T, inumT, is768T
                        row0 = U_PAD + t * 512
                    cols = slice(t * 512, (t + 1) * 512)
                    qn = t % 4

                    # ---- gathers ----
                    fit = p1.tile([128, 320], I16, tag="fit", name="fit")
                    nc.sync.dma_start(out=fit[:], in_=fidx[t])
                    gf = p1.tile([128, 40, 64], F32, tag="gf", name="gf")
                    nc.gpsimd.dma_gather(out_ap=gf[:], in_ap=ftab[:], idxs_ap=fit[:],
                                         num_idxs=5120, num_idxs_reg=5120,
                                         elem_size=64, single_packet=False,
                                         queue_num=qn)
                    tit = p1.tile([128, 768], I16, tag="tit", name="tit")
                    nc.sync.dma_start(out=tit[:], in_=tidx[t])
                    gt = p1.tile([128, 96, 64], F32, tag="gt", name="gt")
                    nc.gpsimd.dma_gather(out_ap=gt[:], in_ap=wordp[:], idxs_ap=tit[:],
                                         num_idxs=12288, num_idxs_reg=12288,
                                         elem_size=64, single_packet=False,
                                         queue_num=(qn + 1) % 4)

                    # ---- bag sums (feat: 10 -> [128,4,64]; text: 8 -> [128,12,64])
                    feat_nm = _bag_sum(nc, p1, gf[:], 4, 10, 64, "f")
                    text_nm = _bag_sum(nc, p1, gt[:], 12, 8, 64, "t")
                    # text: keep only first 32 cols, contiguous [128, 4, 3, 32]
                    text_c = p1.tile([128, 4, 3, 32], F32, tag="textc", name="textc")
                    nc.vector.tensor_copy(
                        out=text_c[:],
                        in_=text_nm[:].rearrange("p (j s) e -> p j s e", s=3)[:, :, :, 0:32])

                    # ---- transposes into feature-major rhs ----
                    rhs_feat = p1r.tile([64, 512], F32, tag="rhs_feat", name="rhs_feat")
                    rhs_text = p1r.tile([96, 512], F32, tag="rhs_text", name="rhs_text")
                    for j in range(4):
                        trf = ps_tr.tile([128, 128], F32, tag="tr", name="trf")
                        nc.tensor.transpose(out=trf[:64, :], in_=feat_nm[:, j, :],
                                            identity=ident[:])
                        nc.vector.tensor_copy(out=rhs_feat[:, j * 128:(j + 1) * 128],
                                              in_=trf[:64, :])
                        trt = ps_tr.tile([128, 128], F32, tag="tr", name="trt")
                        nc.tensor.transpose(out=trt[:96, :], in_=text_c[:, j],
                                            identity=ident[:])
                        nc.vector.tensor_copy(out=rhs_text[:, j * 128:(j + 1) * 128],
                                              in_=trt[:96, :])

                    # ---- dense rhs tiles ----
                    rhs = {"feat": rhs_feat, "text": rhs_text}
                    rhs_id = p1r.tile([64, 512], F32, tag="rhs_id", name="rhs_id")
                    nc.sync.dma_start(out=rhs_id[:], in_=idT[:, cols])
                    rhs["id"] = rhs_id
                    for k, nm in enumerate(["w3a", "w3b", "w3c"]):
                        rr = (128, 128, 44)[k]
                        rt = p1r.tile([rr, 512], F32, tag=f"rhs_{nm}", name=f"rhs_{nm}")
                        nc.sync.dma_start(out=rt[:],
                                          in_=w300T[k * 128:k * 128 + rr, cols])
                        rhs[nm] = rt
                    rhs_num = p1r.tile([10, 512], F32, tag="rhs_num", name="rhs_num")
                    nc.sync.dma_start(out=rhs_num[:], in_=numT[:, cols])
                    rhs["num"] = rhs_num
                    if s768T is not None:
                        for k in range(6):
                            nm = f"s7{'abcdef'[k]}"
                            rt = p1r.tile([128, 512], F32, tag=f"rhs_{nm}",
                                          name=f"rhs_{nm}")
                            nc.sync.dma_start(out=rt[:],
                                              in_=s768T[k * 128:(k + 1) * 128, cols])
                            rhs[nm] = rt

                    # ---- projection matmul (accumulate over segments) ----
                    acc = ps_acc.tile([64, 512], F32, tag="acc", name="acc")
                    for k, (nm, r0, r1) in enumerate(segs):
                        nc.tensor.matmul(acc[:], lhsT=wseg[nm][:], rhs=rhs[nm][:],
                                         start=(k == 0), stop=(k == len(segs) - 1))
                    x_fm = p1.tile([64, 512], F32, tag="x_fm", name="x_fm")
                    nc.vector.tensor_tensor(out=x_fm[:], in0=acc[:],
                                            in1=bseg[:].to_broadcast([64, 512]),
                                            op=mybir.AluOpType.add)
                    pv0 = ps_v0.tile([64, 512], F32, tag="pv0", name="pv0")
                    nc.tensor.matmul(pv0[:], lhsT=wv0_t[:], rhs=x_fm[:],
                                     start=True, stop=True)
                    v0_fm = p1.tile([64, 512], F32, tag="v0_fm", name="v0_fm")
                    nc.vector.tensor_tensor(out=v0_fm[:], in0=pv0[:],
                                            in1=bv0_t[:].to_broadcast([64, 512]),
                                            op=mybir.AluOpType.add)

                    # ---- transpose back to node-major [128,128] and store ----
                    for j in range(4):
                        jc = slice(j * 128, (j + 1) * 128)
                        xv_sb = p1.tile([128, 128], F32, tag="xv_sb", name="xv_sb")
                        trx = ps_tr.tile([128, 128], F32, tag="tr", name="trx")
                        nc.tensor.transpose(out=trx[:, 0:64], in_=x_fm[:, jc],
                                            identity=ident[:64, :64])
                        nc.vector.tensor_copy(out=xv_sb[:, 0:64], in_=trx[:, 0:64])
                        trv = ps_tr.tile([128, 128], F32, tag="tr", name="trv")
                        nc.tensor.transpose(out=trv[:, 0:64], in_=v0_fm[:, jc],
                                            identity=ident[:64, :64])
                        nc.vector.tensor_copy(out=xv_sb[:, 64:128], in_=trv[:, 0:64])
                        nc.sync.dma_start(
                            out=xv_slice[row0 + j * 128: row0 + (j + 1) * 128, :],
                            in_=xv_sb[:])

                for t in range(NT_U):
                    node_tile("u", t)
                for t in range(NT_I):
                    node_tile("i", t)

            # ---------------- all-gather ----------------
            nc.gpsimd.collective_compute(
                "AllGather", mybir.AluOpType.bypass,
                replica_groups=[list(range(NCORES))],
                ins=[xv_slice.opt()], outs=[xv_full.opt()])

            # ---------------- phase 2 ----------------
            with tc.tile_pool(name="p2", bufs=2) as p2, \
                 tc.tile_pool(name="p2b", bufs=1) as p2b, \
                 tc.tile_pool(name="ps2", bufs=2, space="PSUM") as ps2, \
                 tc.tile_pool(name="ps2t", bufs=2, space="PSUM") as ps2t:

                aggh1 = p2b.tile([64, 256], F32)

                for c2 in range(NCH2):
                    it2 = p2.tile([128, 100], I32, tag="it2", name="it2")
                    nc.sync.dma_start(out=it2[:],
                                      in_=nb2idx[c2].rearrange("k p -> p k"))
                    g2 = p2.tile([128, 100, 64], F32, tag="g2", name="g2")
                    for k in range(100):
                        nc.gpsimd.indirect_dma_start(
                            out=g2[:, k, :], out_offset=None, in_=xv_full[:],
                            in_offset=bass.IndirectOffsetOnAxis(
                                ap=it2[:, k:k + 1], axis=0),
                            element_offset=64)
                    agg2_nm = _bag_sum(nc, p2, g2[:], 5, 20, 64, "g2")

                    itx1 = p2.tile([128, 5], I32, tag="itx1", name="itx1")
                    nc.sync.dma_start(out=itx1[:],
                                      in_=x1idx[c2].rearrange("k p -> p k"))
                    gx1 = p2.tile([128, 5, 64], F32, tag="gx1", name="gx1")
                    for k in range(5):
                        nc.gpsimd.indirect_dma_start(
                            out=gx1[:, k, :], out_offset=None, in_=xv_full[:],
                            in_offset=bass.IndirectOffsetOnAxis(
                                ap=itx1[:, k:k + 1], axis=0))

                    x1T = p2.tile([64, 640], F32, tag="x1T", name="x1T")
                    agg2T = p2.tile([64, 640], F32, tag="agg2T", name="agg2T")
                    for j in range(5):
                        col = slice(j * 128, (j + 1) * 128)
                        tra = ps2t.tile([128, 128], F32, tag="tr2", name="tra")
                        nc.tensor.transpose(out=tra[:64, :], in_=agg2_nm[:, j, :],
                                            identity=ident[:])
                        nc.vector.tensor_copy(out=agg2T[:, col], in_=tra[:64, :])
                        trx1 = ps2t.tile([128, 128], F32, tag="tr2", name="trx1")
                        nc.tensor.transpose(out=trx1[:64, :], in_=gx1[:, j, :],
                                            identity=ident[:])
                        nc.vector.tensor_copy(out=x1T[:, col], in_=trx1[:64, :])

                    # h1 = relu(Ww0t.T@x1T + Ww0b.T@agg2T + bw0); v1 = Wv1.T@h1+bv1
                    # per sub-chunk of 320 pairs (= 16 seeds)
                    v1c = p2.tile([64, 640], F32, tag="v1c", name="v1c")
                    for n in range(2):
                        col = slice(n * 320, (n + 1) * 320)
                        ph = ps2.tile([64, 320], F32, tag="ph", name="ph")
                        nc.tensor.matmul(ph[:], lhsT=ww0t_t[:], rhs=x1T[:, col],
                                         start=True, stop=False)
                        nc.tensor.matmul(ph[:], lhsT=ww0b_t[:], rhs=agg2T[:, col],
                                         start=False, stop=True)
                        h1c = p2.tile([64, 320], F32, tag="h1c", name="h1c")
                        nc.scalar.activation(h1c[:], ph[:],
                                             mybir.ActivationFunctionType.Relu,
                                             bias=bw0_t[:])
                        pv = ps2.tile([64, 320], F32, tag="pv", name="pv")
                        nc.tensor.matmul(pv[:], lhsT=wv1_t[:], rhs=h1c[:],
                                         start=True, stop=True)
                        nc.vector.tensor_tensor(
                            out=v1c[:, col], in0=pv[:],
                            in1=bv1_t[:].to_broadcast([64, 320]),
                            op=mybir.AluOpType.add)
                    # agg_h1 for these 32 seeds: sum over s of [64, 32, 20]
                    vv = v1c[:].rearrange("p (b s) -> p b s", s=20)
                    va = p2.tile([64, 32, 10], F32, tag="va", name="va")
                    nc.vector.tensor_tensor(out=va[:], in0=vv[:, :, 0:10],
                                            in1=vv[:, :, 10:20],
                                            op=mybir.AluOpType.add)
                    vb = p2.tile([64, 32, 5], F32, tag="vb", name="vb")
                    nc.vector.tensor_tensor(out=vb[:], in0=va[:, :, 0:5],
                                            in1=va[:, :, 5:10],
                                            op=mybir.AluOpType.add)
                    vc = p2.tile([64, 32, 2], F32, tag="vc", name="vc")
                    nc.vector.tensor_tensor(out=vc[:], in0=vb[:, :, 0:2],
                                            in1=vb[:, :, 2:4],
                                            op=mybir.AluOpType.add)
                    vd = p2.tile([64, 32], F32, tag="vd", name="vd")
                    nc.vector.tensor_tensor(out=vd[:], in0=vc[:, :, 0],
                                            in1=vc[:, :, 1],
                                            op=mybir.AluOpType.add)
                    nc.vector.tensor_tensor(
                        out=aggh1[:, c2 * 32:(c2 + 1) * 32], in0=vd[:],
                        in1=vb[:, :, 4], op=mybir.AluOpType.add)

                # neigh1 in seed order -> agg1
                itn1 = p2b.tile([128, 40], I32)
                nc.sync.dma_start(out=itn1[:], in_=nb1sidx[:].rearrange("k p -> p k"))
                gn1 = p2b.tile([128, 40, 64], F32)
                for k in range(40):
                    nc.gpsimd.indirect_dma_start(
                        out=gn1[:, k, :], out_offset=None, in_=xv_full[:],
                        in_offset=bass.IndirectOffsetOnAxis(
                            ap=itn1[:, k:k + 1], axis=0),
                        element_offset=64)
                agg1_nm = _bag_sum(nc, p2, gn1[:], 2, 20, 64, "n1")
                agg1T = p2b.tile([64, 256], F32)
                # seeds
                its = p2b.tile([128, 2], I32)
                nc.sync.dma_start(out=its[:], in_=seedidx[:].rearrange("k p -> p k"))
                gs = p2b.tile([128, 2, 64], F32)
                for k in range(2):
                    nc.gpsimd.indirect_dma_start(
                        out=gs[:, k, :], out_offset=None, in_=xv_full[:],
                        in_offset=bass.IndirectOffsetOnAxis(
                            ap=its[:, k:k + 1], axis=0))
                x0T = p2b.tile([64, 256], F32)
                for j in range(2):
                    col = slice(j * 128, (j + 1) * 128)
                    tr1 = ps2t.tile([128, 128], F32, tag="tr2", name="tr1")
                    nc.tensor.transpose(out=tr1[:64, :], in_=agg1_nm[:, j, :],
                                        identity=ident[:])
                    nc.vector.tensor_copy(out=agg1T[:, col], in_=tr1[:64, :])
                    tr0 = ps2t.tile([128, 128], F32, tag="tr2", name="tr0")
                    nc.tensor.transpose(out=tr0[:64, :], in_=gs[:, j, :],
                                        identity=ident[:])
                    nc.vector.tensor_copy(out=x0T[:, col], in_=tr0[:64, :])

                # h0 = relu(Ww0t.T@x0T + Ww0b.T@agg1T + bw0)
                ph0 = ps2.tile([64, 256], F32, tag="ph", name="ph0")
                nc.tensor.matmul(ph0[:], lhsT=ww0t_t[:], rhs=x0T[:],
                                 start=True, stop=False)
                nc.tensor.matmul(ph0[:], lhsT=ww0b_t[:], rhs=agg1T[:],
                                 start=False, stop=True)
                h0 = p2b.tile([64, 256], F32)
                nc.scalar.activation(h0[:], ph0[:],
                                     mybir.ActivationFunctionType.Relu, bias=bw0_t[:])

                # out = Ww1t.T@h0 + Ww1b.T@aggh1 + bw1
                po = ps2.tile([64, 256], F32, tag="ph", name="po")
                nc.tensor.matmul(po[:], lhsT=ww1t_t[:], rhs=h0[:],
                                 start=True, stop=False)
                nc.tensor.matmul(po[:], lhsT=ww1b_t[:], rhs=aggh1[:],
                                 start=False, stop=True)
                out_fm = p2b.tile([64, 256], F32)
                nc.vector.tensor_tensor(out=out_fm[:], in0=po[:],
                                        in1=bw1_t[:].to_broadcast([64, 256]),
                                        op=mybir.AluOpType.add)
                out_nm = p2b.tile([128, 2, 64], F32)
                for j in range(2):
                    tro = ps2t.tile([128, 128], F32, tag="tr2", name="tro")
                    nc.tensor.transpose(out=tro[:, 0:64],
                                        in_=out_fm[:, j * 128:(j + 1) * 128],
                                        identity=ident[:64, :64])
                    nc.vector.tensor_copy(out=out_nm[:, j, :], in_=tro[:, 0:64])
                # DRAM row r = p*2+u  (host un-permutes)
                nc.sync.dma_start(
                    out=out[:].rearrange("(p u) e -> p u e", u=2), in_=out_nm[:])

    nc.compile()
    return nc


def _prep_inputs(inputs):
    """Host-side sharding/layout. Returns in_maps (list of 8 dicts)."""
    gi = {k: np.asarray(v) for k, v in inputs.items()}

    def pad_rows(a, n):
        if a.shape[0] == n:
            return a
        pad = np.zeros((n - a.shape[0],) + a.shape[1:], a.dtype)
        return np.concatenate([a, pad], axis=0)

    # ---- weights (shared) ----
    Wpu = gi["Wproj_u"].astype(np.float32)
    Wpi = gi["Wproj_i"].astype(np.float32)
    W_user = np.concatenate([
        Wpu[0:64], Wpu[64:128] / 10.0, Wpu[128:224] / 8.0, Wpu[224:524],
        gi["Wnum_u"].astype(np.float32)], axis=0)
    b_user = (gi["bproj_u"] + gi["bnum_u"]).astype(np.float32)
    W_item = np.concatenate([
        Wpi[0:64], Wpi[64:128] / 10.0, Wpi[128:224] / 8.0, Wpi[224:1292],
        gi["Wnum_i"].astype(np.float32)], axis=0)
    b_item = (gi["bproj_i"] + gi["bnum_i"]).astype(np.float32)
    shared = {
        "W_user": np.ascontiguousarray(W_user),
        "b_user": b_user,
        "W_item": np.ascontiguousarray(W_item),
        "b_item": b_item,
        "Wv0": np.ascontiguousarray(gi["W_v"][0] / 20.0).astype(np.float32),
        "bv0": (gi["b_v"][0] / 20.0).astype(np.float32),
        "Wv1": np.ascontiguousarray(gi["W_v"][1] / 20.0).astype(np.float32),
        "bv1": (gi["b_v"][1] / 20.0).astype(np.float32),
        "Ww0t": np.ascontiguousarray(gi["W_w"][0][:64]).astype(np.float32),
        "Ww0b": np.ascontiguousarray(gi["W_w"][0][64:]).astype(np.float32),
        "bw0": gi["b_w"][0].astype(np.float32),
        "Ww1t": np.ascontiguousarray(gi["W_w"][1][:64]).astype(np.float32),
        "Ww1b": np.ascontiguousarray(gi["W_w"][1][64:]).astype(np.float32),
        "bw1": gi["b_w"][1].astype(np.float32),
        "ufe": np.ascontiguousarray(gi["user_feat_emb"]).astype(np.float32),
        "ife": np.ascontiguousarray(gi["item_feat_emb"]).astype(np.float32),
        "wordp": np.concatenate(
            [gi["word_emb"].astype(np.float32),
             np.zeros((VOCAB, 32), np.float32)], axis=1),
    }

    seeds_r = _remap_nodes(gi["seeds"])
    nb1_r = _remap_nodes(gi["neigh1"])
    nb2_r = _remap_nodes(gi["neigh2"])

    in_maps = []
    for c in range(NCORES):
        us = slice(c * U_PC, (c + 1) * U_PC)
        isl = slice(c * I_PC, (c + 1) * I_PC)
        m = dict(shared)
        m["uidT"] = np.ascontiguousarray(
            pad_rows(gi["user_id_emb"][us], U_PAD).T).astype(np.float32)
        m["uw300T"] = np.ascontiguousarray(
            pad_rows(gi["user_word300"][us], U_PAD).T).astype(np.float32)
        m["unumT"] = np.ascontiguousarray(
            pad_rows(gi["user_numeric"][us], U_PAD).T).astype(np.float32)
        m["iidT"] = np.ascontiguousarray(
            pad_rows(gi["item_id_emb"][isl], I_PAD).T).astype(np.float32)
        m["iw300T"] = np.ascontiguousarray(
            pad_rows(gi["item_word300"][isl], I_PAD).T).astype(np.float32)
        m["is768T"] = np.ascontiguousarray(
            pad_rows(gi["item_sent768"][isl], I_PAD).T).astype(np.float32)
        m["inumT"] = np.ascontiguousarray(
            pad_rows(gi["item_numeric"][isl], I_PAD).T).astype(np.float32)

        # phase-1 gather indices
        uf = pad_rows(np.asarray(gi["user_feat_idx"][us]), U_PAD)  # [U_PAD, 10]
        # flat[t, (j*10+f)*128 + p] = uf[t*512 + j*128 + p, f]
        uff = uf.reshape(NT_U, 4, 128, 10).transpose(0, 1, 3, 2).reshape(NT_U, 5120)
        m["ufidx"] = _wrap_idx(uff)
        ut = pad_rows(np.asarray(gi["user_text_idx"][us]), U_PAD)  # [U_PAD, 3, 8]
        utf = ut.reshape(NT_U, 4, 128, 24).transpose(0, 1, 3, 2).reshape(NT_U, 12288)
        m["utidx"] = _wrap_idx(utf)
        if_ = pad_rows(np.asarray(gi["item_feat_idx"][isl]), I_PAD)
        iff = if_.reshape(NT_I, 4, 128, 10).transpose(0, 1, 3, 2).reshape(NT_I, 5120)
        m["ifidx"] = _wrap_idx(iff)
        it_ = pad_rows(np.asarray(gi["item_text_idx"][isl]), I_PAD)
        itf = it_.reshape(NT_I, 4, 128, 24).transpose(0, 1, 3, 2).reshape(NT_I, 12288)
        m["itidx"] = _wrap_idx(itf)

        # phase-2 indices
        bs = slice(c * B_PC, (c + 1) * B_PC)
        nb2c = nb2_r[bs].reshape(PAIRS, 20)      # pair-major [5120, 20]
        # [c2, k=j*20+t, p] = nb2c[c2*640 + j*128 + p, t]
        nb2t = nb2c.reshape(NCH2, 5, 128, 20).transpose(0, 1, 3, 2)  # [8,5,20,128]
        m["nb2idx"] = np.ascontiguousarray(
            nb2t.reshape(NCH2, 100, 128)).astype(np.int32)
        nb1c = nb1_r[bs].reshape(PAIRS)          # [5120]
        m["x1idx"] = np.ascontiguousarray(
            nb1c.reshape(NCH2, 5, 128)).astype(np.int32)
        # seed order: [k=u*20+t, p] = nb1[u*128+p, t]
        nb1s = nb1_r[bs].reshape(2, 128, 20).transpose(0, 2, 1)  # [2, 20, 128]
        m["nb1sidx"] = np.ascontiguousarray(
            nb1s.reshape(40, 128)).astype(np.int32)
        m["seedidx"] = np.ascontiguousarray(
            seeds_r[bs].reshape(2, 128)).astype(np.int32)
        in_maps.append(m)
    return in_maps


def kernel(**inputs) -> np.ndarray:
    global LAST_RESULT, _CACHED
    if _CACHED is None:
        _CACHED = _build_program()
    nc = _CACHED
    in_maps = _prep_inputs(inputs)
    trace = bool(int(os.environ.get("KERNEL_TRACE", "0")))
    res = run_bass_kernel_spmd(nc, in_maps, core_ids=list(range(NCORES)),
                               trace=trace)
    LAST_RESULT = res
    out = np.empty((B, 64), np.float32)
    for c in range(NCORES):
        oc = res.results[c]["out"].reshape(128, 2, 64)
        out[c * B_PC:(c + 1) * B_PC] = (
            oc.transpose(1, 0, 2).reshape(B_PC, 64))
    return out


# revision 11
# speedup vs baseline: 1.4337x; 1.4337x over previous
"""Distributed Bass kernel for nn_ASAGE (GraphSAGE over a two-tower user/item
graph) on 8 TRN2 NeuronCores.

Strategy:
  Phase 1 (node embeddings, node-sharded): each core computes x = proj(feats)
  and v0 = x @ (W_v0/20) for its 1/8 slice of users+items. Embedding-bag
  lookups (feature bags, text tokens) run as big int16 `dma_gather`s from
  DRAM tables; dense features are host-pre-transposed so they DMA straight
  into feature-major matmul operands.
  AllGather of [x||v0] (one [19456,128] f32 tile per core -> [155648,128]).
  Phase 2 (data-parallel over seeds): per-core 256 seeds; v0 rows for
  neighbor aggregation gathered via indirect DMA (int32 indices); two
  GraphSAGE layers as feature-major matmuls.

Host-side work is restricted to layout transformations: slicing/sharding,
integer index remapping/reordering, weight concat + folding of the constant
bag-mean scales (1/10, 1/8, 1/20) into the corresponding weight rows.
"""
import os
import sys
import types

sys.path.insert(0, "/opt/trn_rl_repo")

import numpy as np


def _ensure_axon_hooks():
    """Provide antenv.axon_hooks + register the NTFF profile hook (the agent
    image's antenv lacks it). Needed only for trace=True timing runs."""
    try:
        import antenv
    except ImportError:
        return
    if "antenv.axon_hooks" in sys.modules:
        return
    mod = types.ModuleType("antenv.axon_hooks")
    mod._HOOK = None

    def set_axon_ntff_profile_hook(hook, _mod=mod):
        _mod._HOOK = hook

    def get_axon_ntff_profile_hook(_mod=mod):
        return _mod._HOOK

    mod.set_axon_ntff_profile_hook = set_axon_ntff_profile_hook
    mod.get_axon_ntff_profile_hook = get_axon_ntff_profile_hook
    sys.modules["antenv.axon_hooks"] = mod
    antenv.axon_hooks = mod
    try:
        from trn_agent_boot.trn_boot import _ntff_profile_via_ctypes

        mod.set_axon_ntff_profile_hook(
            _ntff_profile_via_ctypes("/opt/axon/libaxon_pjrt.so")
        )
    except Exception:
        pass


_ensure_axon_hooks()

import concourse.bass as bass
import concourse.bacc as bacc
import concourse.tile as tile
import concourse.mybir as mybir
from concourse.bass_utils import run_bass_kernel_spmd
from concourse.masks import make_identity

# ---- problem constants (hardcoded per contract) ----
N_USER = 100000
M_ITEM = 50000
D = 64
WD = 32
B = 2048
S = 20
VOCAB = 20000
NCORES = 8

U_PC = N_USER // NCORES          # 12500 real users per core
I_PC = M_ITEM // NCORES          # 6250 real items per core
U_PAD = 12800                    # 25 tiles of 512
I_PAD = 6656                     # 13 tiles of 512
NT_U = U_PAD // 512
NT_I = I_PAD // 512
NODES_PC = U_PAD + I_PAD         # 19456
XV_ROWS = NODES_PC * NCORES      # 155648
B_PC = B // NCORES               # 256
PAIRS = B_PC * S                 # 5120
NCH2 = 8                         # neigh2 chunks
PAIRS_CH = PAIRS // NCH2         # 640 = 5 * 128
CAP2 = 1920                      # per-(chunk, region) stage-a bucket capacity

F32 = mybir.dt.float32
I32 = mybir.dt.int32
I16 = mybir.dt.int16

LAST_RESULT = None  # BassKernelResults of the most recent run (for test.py)

_CACHED = None  # (nc, static_in_map) — compile once per process


def _remap_nodes(idx):
    """Map reference node ids (users 0..1e5, items 1e5..1.5e5) to AG row ids."""
    idx = np.asarray(idx, dtype=np.int64)
    user = idx < N_USER
    out = np.empty_like(idx)
    u = idx[user]
    out[user] = (u // U_PC) * NODES_PC + (u % U_PC)
    m = idx[~user] - N_USER
    out[~user] = (m // I_PC) * NODES_PC + U_PAD + (m % I_PC)
    return out.astype(np.int32)


def _wrap_idx(flat):
    """dma_gather int16 index layout: [128, n/16], idx for element i at
    partition i%16 (replicated to all 8 16-partition groups), free slot i//16."""
    flat = np.asarray(flat, dtype=np.int16)
    n = flat.shape[-1]
    lead = flat.shape[:-1]
    w = flat.reshape(*lead, n // 16, 16)
    w = np.swapaxes(w, -1, -2)  # [..., 16, n//16]
    return np.tile(w, (1,) * len(lead) + (8, 1)).astype(np.int16)


def _bag_sum(nc, pool, g, nsub, bag, e, name):
    """Sum over the `bag` axis of g viewed as [128, nsub, bag, e].
    Returns a contiguous [128, nsub, e] tile. bag in (8, 10, 20)."""
    v = g.rearrange("p (j b) e -> p j b e", b=bag)
    if bag == 8:
        a1 = pool.tile([128, nsub, 4, e], F32, tag=f"{name}a1", name=f"{name}a1")
        nc.vector.tensor_tensor(out=a1[:], in0=v[:, :, 0:4, :], in1=v[:, :, 4:8, :],
                                op=mybir.AluOpType.add)
        a2 = pool.tile([128, nsub, 2, e], F32, tag=f"{name}a2", name=f"{name}a2")
        nc.vector.tensor_tensor(out=a2[:], in0=a1[:, :, 0:2, :], in1=a1[:, :, 2:4, :],
                                op=mybir.AluOpType.add)
        a3 = pool.tile([128, nsub, e], F32, tag=f"{name}a3", name=f"{name}a3")
        nc.vector.tensor_tensor(out=a3[:], in0=a2[:, :, 0, :], in1=a2[:, :, 1, :],
                                op=mybir.AluOpType.add)
        return a3
    if bag == 10:
        s1 = pool.tile([128, nsub, 5, e], F32, tag=f"{name}s1", name=f"{name}s1")
        nc.vector.tensor_tensor(out=s1[:], in0=v[:, :, 0:5, :], in1=v[:, :, 5:10, :],
                                op=mybir.AluOpType.add)
        s2 = pool.tile([128, nsub, 2, e], F32, tag=f"{name}s2", name=f"{name}s2")
        nc.vector.tensor_tensor(out=s2[:], in0=s1[:, :, 0:2, :], in1=s1[:, :, 2:4, :],
                                op=mybir.AluOpType.add)
        s3 = pool.tile([128, nsub, e], F32, tag=f"{name}s3", name=f"{name}s3")
        nc.vector.tensor_tensor(out=s3[:], in0=s2[:, :, 0, :], in1=s2[:, :, 1, :],
                                op=mybir.AluOpType.add)
        s4 = pool.tile([128, nsub, e], F32, tag=f"{name}s4", name=f"{name}s4")
        nc.vector.tensor_tensor(out=s4[:], in0=s3[:], in1=s1[:, :, 4, :],
                                op=mybir.AluOpType.add)
        return s4
    assert bag == 20
    a = pool.tile([128, nsub, 10, e], F32, tag=f"{name}a", name=f"{name}a")
    nc.vector.tensor_tensor(out=a[:], in0=v[:, :, 0:10, :], in1=v[:, :, 10:20, :],
                            op=mybir.AluOpType.add)
    b5 = pool.tile([128, nsub, 5, e], F32, tag=f"{name}b", name=f"{name}b")
    nc.vector.tensor_tensor(out=b5[:], in0=a[:, :, 0:5, :], in1=a[:, :, 5:10, :],
                            op=mybir.AluOpType.add)
    c2 = pool.tile([128, nsub, 2, e], F32, tag=f"{name}c", name=f"{name}c")
    nc.vector.tensor_tensor(out=c2[:], in0=b5[:, :, 0:2, :], in1=b5[:, :, 2:4, :],
                            op=mybir.AluOpType.add)
    d = pool.tile([128, nsub, e], F32, tag=f"{name}d", name=f"{name}d")
    nc.vector.tensor_tensor(out=d[:], in0=c2[:, :, 0, :], in1=c2[:, :, 1, :],
                            op=mybir.AluOpType.add)
    ee = pool.tile([128, nsub, e], F32, tag=f"{name}e", name=f"{name}e")
    nc.vector.tensor_tensor(out=ee[:], in0=d[:], in1=b5[:, :, 4, :],
                            op=mybir.AluOpType.add)
    return ee


def _build_program():
    nc = bacc.Bacc("TRN2", target_bir_lowering=False, debug=False,
                   num_devices=NCORES, num_swdge_queues=4)
    dt = nc.dram_tensor

    # dense (host pre-transposed, padded) per-core inputs
    uidT = dt("uidT", [64, U_PAD], F32, kind="ExternalInput")
    uw300T = dt("uw300T", [300, U_PAD], F32, kind="ExternalInput")
    unumT = dt("unumT", [10, U_PAD], F32, kind="ExternalInput")
    iidT = dt("iidT", [64, I_PAD], F32, kind="ExternalInput")
    iw300T = dt("iw300T", [300, I_PAD], F32, kind="ExternalInput")
    is768T = dt("is768T", [768, I_PAD], F32, kind="ExternalInput")
    inumT = dt("inumT", [10, I_PAD], F32, kind="ExternalInput")

    # gather tables (replicated)
    ufe = dt("ufe", [30000, 64], F32, kind="ExternalInput")
    ife = dt("ife", [30000, 64], F32, kind="ExternalInput")
    wordp = dt("wordp", [VOCAB, 64], F32, kind="ExternalInput")  # padded 32->64

    # gather indices
    ufidx = dt("ufidx", [NT_U, 128, 320], I16, kind="ExternalInput")
    utidx = dt("utidx", [NT_U, 128, 768], I16, kind="ExternalInput")
    ifidx = dt("ifidx", [NT_I, 128, 320], I16, kind="ExternalInput")
    itidx = dt("itidx", [NT_I, 128, 768], I16, kind="ExternalInput")
    nb2aidx = dt("nb2aidx", [NCH2, 8, 128, CAP2 // 16], I16, kind="ExternalInput")
    nb2bidx = dt("nb2bidx", [NCH2, 128, 800], I16, kind="ExternalInput")
    x1idx = dt("x1idx", [NCH2, 5, 128], I32, kind="ExternalInput")
    nb1sidx = dt("nb1sidx", [40, 128], I32, kind="ExternalInput")
    seedidx = dt("seedidx", [2, 128], I32, kind="ExternalInput")

    # weights (replicated; host-packed, scales folded)
    W_user = dt("W_user", [534, 64], F32, kind="ExternalInput")
    b_user = dt("b_user", [64], F32, kind="ExternalInput")
    W_item = dt("W_item", [1302, 64], F32, kind="ExternalInput")
    b_item = dt("b_item", [64], F32, kind="ExternalInput")
    Wv0 = dt("Wv0", [64, 64], F32, kind="ExternalInput")     # /20 folded
    bv0 = dt("bv0", [64], F32, kind="ExternalInput")
    Wv1 = dt("Wv1", [64, 64], F32, kind="ExternalInput")     # /20 folded
    bv1 = dt("bv1", [64], F32, kind="ExternalInput")
    Ww0t = dt("Ww0t", [64, 64], F32, kind="ExternalInput")
    Ww0b = dt("Ww0b", [64, 64], F32, kind="ExternalInput")
    bw0 = dt("bw0", [64], F32, kind="ExternalInput")
    Ww1t = dt("Ww1t", [64, 64], F32, kind="ExternalInput")
    Ww1b = dt("Ww1b", [64, 64], F32, kind="ExternalInput")
    bw1 = dt("bw1", [64], F32, kind="ExternalInput")

    out = dt("out", [B_PC, 64], F32, kind="ExternalOutput")

    # user/item segment row-ranges in W_user/W_item
    useg = [("id", 0, 64), ("feat", 64, 128), ("text", 128, 224),
            ("w3a", 224, 352), ("w3b", 352, 480), ("w3c", 480, 524),
            ("num", 524, 534)]
    iseg = [("id", 0, 64), ("feat", 64, 128), ("text", 128, 224),
            ("w3a", 224, 352), ("w3b", 352, 480), ("w3c", 480, 524),
            ("s7a", 524, 652), ("s7b", 652, 780), ("s7c", 780, 908),
            ("s7d", 908, 1036), ("s7e", 1036, 1164), ("s7f", 1164, 1292),
            ("num", 1292, 1302)]

    qctr = [0]

    def next_q():
        q = qctr[0] % 4
        qctr[0] += 1
        return q

    with tile.TileContext(nc) as tc:
        with tc.tile_pool(name="const", bufs=1) as constp, \
             tc.tile_pool(name="dram", bufs=1, space="DRAM") as dram:
            ident = constp.tile([128, 128], F32)
            make_identity(nc, ident[:])

            def load_w(t, rows=64):
                w = constp.tile([rows, 64], F32, name=f"w_{t.name}", uniquify=True)
                nc.sync.dma_start(out=w[:], in_=t[:])
                return w

            def load_b(t):
                b_ = constp.tile([64, 1], F32, name=f"b_{t.name}", uniquify=True)
                nc.sync.dma_start(out=b_[:], in_=t[:, None])
                return b_

            wu = {}
            for nm, r0, r1 in useg:
                w = constp.tile([r1 - r0, 64], F32, name=f"wu_{nm}")
                nc.sync.dma_start(out=w[:], in_=W_user[r0:r1, :])
                wu[nm] = w
            wi = {}
            for nm, r0, r1 in iseg:
                w = constp.tile([r1 - r0, 64], F32, name=f"wi_{nm}")
                nc.sync.dma_start(out=w[:], in_=W_item[r0:r1, :])
                wi[nm] = w
            bu_t = load_b(b_user)
            bi_t = load_b(b_item)
            wv0_t = load_w(Wv0)
            bv0_t = load_b(bv0)
            wv1_t = load_w(Wv1)
            bv1_t = load_b(bv1)
            ww0t_t = load_w(Ww0t)
            ww0b_t = load_w(Ww0b)
            bw0_t = load_b(bw0)
            ww1t_t = load_w(Ww1t)
            ww1b_t = load_w(Ww1b)
            bw1_t = load_b(bw1)

            xv_slice = dram.tile([NODES_PC, 128], F32)
            xv_full = dram.tile([XV_ROWS, 128], F32, addr_space="Shared")

            # ---------------- phase 1 ----------------
            with tc.tile_pool(name="p1", bufs=2) as p1, \
                 tc.tile_pool(name="p1r", bufs=2) as p1r, \
                 tc.tile_pool(name="ps_acc", bufs=2, space="PSUM") as ps_acc, \
                 tc.tile_pool(name="ps_tr", bufs=2, space="PSUM") as ps_tr, \
                 tc.tile_pool(name="ps_v0", bufs=1, space="PSUM") as ps_v0:

                def node_tile(kind, t):
                    if kind == "u":
                        segs, wseg, bseg = useg, wu, bu_t
                        fidx, tidx, ftab = ufidx, utidx, ufe
                        idT, w300T, numT, s768T = uidT, uw300T, unumT, None
                        row0 = t * 512
                    else:
                        segs, wseg, bseg = iseg, wi, bi_t
                        fidx, tidx, ftab = ifidx, itidx, ife
                        idT, w300T, numT, s768T = iidT, iw300T, inumT, is768T
                        row0 = U_PAD + t * 512
                    cols = slice(t * 512, (t + 1) * 512)

                    # ---- gathers (split for 4-queue concurrency) ----
                    fit = p1.tile([128, 320], I16, tag="fit", name="fit")
                    nc.sync.dma_start(out=fit[:], in_=fidx[t])
                    gf = p1.tile([128, 40, 64], F32, tag="gf", name="gf")
                    nc.gpsimd.dma_gather(out_ap=gf[:], in_ap=ftab[:], idxs_ap=fit[:],
                                         num_idxs=5120, num_idxs_reg=5120,
                                         elem_size=64, single_packet=False,
                                         queue_num=next_q())
                    tit = p1.tile([128, 768], I16, tag="tit", name="tit")
                    nc.sync.dma_start(out=tit[:], in_=tidx[t])
                    gt = p1.tile([128, 96, 64], F32, tag="gt", name="gt")
                    # three 4096-idx sub-gathers into thirds of gt; idx sub-slab
                    # s covers elements [s*4096, (s+1)*4096) = blocks 32s..32s+32
                    for sgi in range(3):
                        nc.gpsimd.dma_gather(
                            out_ap=gt[:, sgi * 32:(sgi + 1) * 32, :],
                            in_ap=wordp[:],
                            idxs_ap=tit[:, sgi * 256:(sgi + 1) * 256],
                            num_idxs=4096, num_idxs_reg=4096,
                            elem_size=64, single_packet=False,
                            queue_num=next_q())

                    # ---- bag sums (feat: 10 -> [128,4,64]; text: 8 -> [128,12,64])
                    feat_nm = _bag_sum(nc, p1, gf[:], 4, 10, 64, "f")
                    text_nm = _bag_sum(nc, p1, gt[:], 12, 8, 64, "t")
                    # text: keep only first 32 cols, contiguous [128, 4, 3, 32]
                    text_c = p1.tile([128, 4, 3, 32], F32, tag="textc", name="textc")
                    nc.vector.tensor_copy(
                        out=text_c[:],
                        in_=text_nm[:].rearrange("p (j s) e -> p j s e", s=3)[:, :, :, 0:32])

                    # ---- transposes into feature-major rhs ----
                    rhs_feat = p1r.tile([64, 512], F32, tag="rhs_feat", name="rhs_feat")
                    rhs_text = p1r.tile([96, 512], F32, tag="rhs_text", name="rhs_text")
                    for j in range(4):
                        trf = ps_tr.tile([128, 128], F32, tag="tr", name="trf")
                        nc.tensor.transpose(out=trf[:64, :], in_=feat_nm[:, j, :],
                                            identity=ident[:])
                        nc.vector.tensor_copy(out=rhs_feat[:, j * 128:(j + 1) * 128],
                                              in_=trf[:64, :])
                        trt = ps_tr.tile([128, 128], F32, tag="tr", name="trt")
                        nc.tensor.transpose(out=trt[:96, :], in_=text_c[:, j],
                                            identity=ident[:])
                        nc.vector.tensor_copy(out=rhs_text[:, j * 128:(j + 1) * 128],
                                              in_=trt[:96, :])

                    # ---- dense rhs tiles ----
                    rhs = {"feat": rhs_feat, "text": rhs_text}
                    rhs_id = p1r.tile([64, 512], F32, tag="rhs_id", name="rhs_id")
                    nc.sync.dma_start(out=rhs_id[:], in_=idT[:, cols])
                    rhs["id"] = rhs_id
                    for k, nm in enumerate(["w3a", "w3b", "w3c"]):
                        rr = (128, 128, 44)[k]
                        rt = p1r.tile([rr, 512], F32, tag=f"rhs_{nm}", name=f"rhs_{nm}")
                        nc.sync.dma_start(out=rt[:],
                                          in_=w300T[k * 128:k * 128 + rr, cols])
                        rhs[nm] = rt
                    rhs_num = p1r.tile([10, 512], F32, tag="rhs_num", name="rhs_num")
                    nc.sync.dma_start(out=rhs_num[:], in_=numT[:, cols])
                    rhs["num"] = rhs_num
                    if s768T is not None:
                        for k in range(6):
                            nm = f"s7{'abcdef'[k]}"
                            rt = p1r.tile([128, 512], F32, tag=f"rhs_{nm}",
                                          name=f"rhs_{nm}")
                            nc.sync.dma_start(out=rt[:],
                                              in_=s768T[k * 128:(k + 1) * 128, cols])
                            rhs[nm] = rt

                    # ---- projection matmul (accumulate over segments) ----
                    acc = ps_acc.tile([64, 512], F32, tag="acc", name="acc")
                    for k, (nm, r0, r1) in enumerate(segs):
                        nc.tensor.matmul(acc[:], lhsT=wseg[nm][:], rhs=rhs[nm][:],
                                         start=(k == 0), stop=(k == len(segs) - 1))
                    x_fm = p1.tile([64, 512], F32, tag="x_fm", name="x_fm")
                    nc.vector.tensor_tensor(out=x_fm[:], in0=acc[:],
                                            in1=bseg[:].to_broadcast([64, 512]),
                                            op=mybir.AluOpType.add)
                    pv0 = ps_v0.tile([64, 512], F32, tag="pv0", name="pv0")
                    nc.tensor.matmul(pv0[:], lhsT=wv0_t[:], rhs=x_fm[:],
                                     start=True, stop=True)
                    v0_fm = p1.tile([64, 512], F32, tag="v0_fm", name="v0_fm")
                    nc.vector.tensor_tensor(out=v0_fm[:], in0=pv0[:],
                                            in1=bv0_t[:].to_broadcast([64, 512]),
                                            op=mybir.AluOpType.add)

                    # ---- transpose back to node-major [128,128] and store ----
                    for j in range(4):
                        jc = slice(j * 128, (j + 1) * 128)
                        xv_sb = p1.tile([128, 128], F32, tag="xv_sb", name="xv_sb")
                        trx = ps_tr.tile([128, 128], F32, tag="tr", name="trx")
                        nc.tensor.transpose(out=trx[:, 0:64], in_=x_fm[:, jc],
                                            identity=ident[:64, :64])
                        nc.tensor.transpose(out=trx[:, 64:128], in_=v0_fm[:, jc],
                                            identity=ident[:64, :64])
                        nc.vector.tensor_copy(out=xv_sb[:], in_=trx[:])
                        nc.sync.dma_start(
                            out=xv_slice[row0 + j * 128: row0 + (j + 1) * 128, :],
                            in_=xv_sb[:])

                for t in range(NT_U):
                    node_tile("u", t)
                for t in range(NT_I):
                    node_tile("i", t)

            # ---------------- all-gather ----------------
            nc.gpsimd.collective_compute(
                "AllGather", mybir.AluOpType.bypass,
                replica_groups=[list(range(NCORES))],
                ins=[xv_slice.opt()], outs=[xv_full.opt()])

            # ---------------- phase 2 ----------------
            with tc.tile_pool(name="p2", bufs=2) as p2, \
                 tc.tile_pool(name="p2b", bufs=1) as p2b, \
                 tc.tile_pool(name="ps2", bufs=2, space="PSUM") as ps2, \
                 tc.tile_pool(name="ps2t", bufs=2, space="PSUM") as ps2t:

                aggh1 = p2b.tile([64, 256], F32)

                seqs = [dram.tile([8 * CAP2, 64], F32, name=f"seq{i}")
                        for i in range(NCH2)]
                for c2 in range(NCH2):
                    # stage-a: region-bucketed gathers xv_full -> seq (v0 half)
                    for o in range(8):
                        ia = p2.tile([128, CAP2 // 16], I16, tag="ia", name="ia")
                        nc.sync.dma_start(out=ia[:], in_=nb2aidx[c2, o])
                        ga = p2.tile([128, CAP2 // 128, 64], F32, tag="ga",
                                     name="ga")
                        nc.gpsimd.dma_gather(
                            out_ap=ga[:],
                            in_ap=xv_full[o * NODES_PC:(o + 1) * NODES_PC, 64:128],
                            idxs_ap=ia[:], num_idxs=CAP2, num_idxs_reg=CAP2,
                            elem_size=64, elem_step=128, single_packet=False,
                            queue_num=next_q())
                        nc.sync.dma_start(
                            out=seqs[c2][o * CAP2:(o + 1) * CAP2, :].rearrange(
                                "(b p) e -> p b e", p=128),
                            in_=ga[:])
                    # stage-b: permute into reduce layout
                    ib = p2.tile([128, 800], I16, tag="ib", name="ib")
                    nc.sync.dma_start(out=ib[:], in_=nb2bidx[c2])
                    g2 = p2.tile([128, 100, 64], F32, tag="g2", name="g2")
                    for sgi in range(4):
                        nc.gpsimd.dma_gather(
                            out_ap=g2[:, sgi * 25:(sgi + 1) * 25, :],
                            in_ap=seqs[c2][:],
                            idxs_ap=ib[:, sgi * 200:(sgi + 1) * 200],
                            num_idxs=3200, num_idxs_reg=3200,
                            elem_size=64, single_packet=False,
                            queue_num=next_q())
                    agg2_nm = _bag_sum(nc, p2, g2[:], 5, 20, 64, "g2")

                    itx1 = p2.tile([128, 5], I32, tag="itx1", name="itx1")
                    nc.sync.dma_start(out=itx1[:],
                                      in_=x1idx[c2].rearrange("k p -> p k"))
                    gx1 = p2.tile([128, 5, 64], F32, tag="gx1", name="gx1")
                    for k in range(5):
                        nc.gpsimd.indirect_dma_start(
                            out=gx1[:, k, :], out_offset=None, in_=xv_full[:],
                            in_offset=bass.IndirectOffsetOnAxis(
                                ap=itx1[:, k:k + 1], axis=0))

                    x1T = p2.tile([64, 640], F32, tag="x1T", name="x1T")
                    agg2T = p2.tile([64, 640], F32, tag="agg2T", name="agg2T")
                    for j in range(5):
                        col = slice(j * 128, (j + 1) * 128)
                        tra = ps2t.tile([128, 128], F32, tag="tr2", name="tra")
                        nc.tensor.transpose(out=tra[:64, :], in_=agg2_nm[:, j, :],
                                            identity=ident[:])
                        nc.vector.tensor_copy(out=agg2T[:, col], in_=tra[:64, :])
                        trx1 = ps2t.tile([128, 128], F32, tag="tr2", name="trx1")
                        nc.tensor.transpose(out=trx1[:64, :], in_=gx1[:, j, :],
                                            identity=ident[:])
                        nc.vector.tensor_copy(out=x1T[:, col], in_=trx1[:64, :])

                    # h1 = relu(Ww0t.T@x1T + Ww0b.T@agg2T + bw0); v1 = Wv1.T@h1+bv1
                    # per sub-chunk of 320 pairs (= 16 seeds)
                    v1c = p2.tile([64, 640], F32, tag="v1c", name="v1c")
                    for n in range(2):
                        col = slice(n * 320, (n + 1) * 320)
                        ph = ps2.tile([64, 320], F32, tag="ph", name="ph")
                        nc.tensor.matmul(ph[:], lhsT=ww0t_t[:], rhs=x1T[:, col],
                                         start=True, stop=False)
                        nc.tensor.matmul(ph[:], lhsT=ww0b_t[:], rhs=agg2T[:, col],
                                         start=False, stop=True)
                        h1c = p2.tile([64, 320], F32, tag="h1c", name="h1c")
                        nc.scalar.activation(h1c[:], ph[:],
                                             mybir.ActivationFunctionType.Relu,
                                             bias=bw0_t[:])
                        pv = ps2.tile([64, 320], F32, tag="pv", name="pv")
                        nc.tensor.matmul(pv[:], lhsT=wv1_t[:], rhs=h1c[:],
                                         start=True, stop=True)
                        nc.vector.tensor_tensor(
                            out=v1c[:, col], in0=pv[:],
                            in1=bv1_t[:].to_broadcast([64, 320]),
                            op=mybir.AluOpType.add)
                    # agg_h1 for these 32 seeds: sum over s of [64, 32, 20]
                    vv = v1c[:].rearrange("p (b s) -> p b s", s=20)
                    va = p2.tile([64, 32, 10], F32, tag="va", name="va")
                    nc.vector.tensor_tensor(out=va[:], in0=vv[:, :, 0:10],
                                            in1=vv[:, :, 10:20],
                                            op=mybir.AluOpType.add)
                    vb = p2.tile([64, 32, 5], F32, tag="vb", name="vb")
                    nc.vector.tensor_tensor(out=vb[:], in0=va[:, :, 0:5],
                                            in1=va[:, :, 5:10],
                                            op=mybir.AluOpType.add)
                    vc = p2.tile([64, 32, 2], F32, tag="vc", name="vc")
                    nc.vector.tensor_tensor(out=vc[:], in0=vb[:, :, 0:2],
                                            in1=vb[:, :, 2:4],
                                            op=mybir.AluOpType.add)
                    vd = p2.tile([64, 32], F32, tag="vd", name="vd")
                    nc.vector.tensor_tensor(out=vd[:], in0=vc[:, :, 0],
                                            in1=vc[:, :, 1],
                                            op=mybir.AluOpType.add)
                    nc.vector.tensor_tensor(
                        out=aggh1[:, c2 * 32:(c2 + 1) * 32], in0=vd[:],
                        in1=vb[:, :, 4], op=mybir.AluOpType.add)

                # neigh1 in seed order -> agg1
                itn1 = p2b.tile([128, 40], I32)
                nc.sync.dma_start(out=itn1[:], in_=nb1sidx[:].rearrange("k p -> p k"))
                gn1 = p2b.tile([128, 40, 64], F32)
                for k in range(40):
                    nc.gpsimd.indirect_dma_start(
                        out=gn1[:, k, :], out_offset=None, in_=xv_full[:],
                        in_offset=bass.IndirectOffsetOnAxis(
                            ap=itn1[:, k:k + 1], axis=0),
                        element_offset=64)
                agg1_nm = _bag_sum(nc, p2, gn1[:], 2, 20, 64, "n1")
                agg1T = p2b.tile([64, 256], F32)
                # seeds
                its = p2b.tile([128, 2], I32)
                nc.sync.dma_start(out=its[:], in_=seedidx[:].rearrange("k p -> p k"))
                gs = p2b.tile([128, 2, 64], F32)
                for k in range(2):
                    nc.gpsimd.indirect_dma_start(
                        out=gs[:, k, :], out_offset=None, in_=xv_full[:],
                        in_offset=bass.IndirectOffsetOnAxis(
                            ap=its[:, k:k + 1], axis=0))
                x0T = p2b.tile([64, 256], F32)
                for j in range(2):
                    col = slice(j * 128, (j + 1) * 128)
                    tr1 = ps2t.tile([128, 128], F32, tag="tr2", name="tr1")
                    nc.tensor.transpose(out=tr1[:64, :], in_=agg1_nm[:, j, :],
                                        identity=ident[:])
                    nc.vector.tensor_copy(out=agg1T[:, col], in_=tr1[:64, :])
                    tr0 = ps2t.tile([128, 128], F32, tag="tr2", name="tr0")
                    nc.tensor.transpose(out=tr0[:64, :], in_=gs[:, j, :],
                                        identity=ident[:])
                    nc.vector.tensor_copy(out=x0T[:, col], in_=tr0[:64, :])

                # h0 = relu(Ww0t.T@x0T + Ww0b.T@agg1T + bw0)
                ph0 = ps2.tile([64, 256], F32, tag="ph", name="ph0")
                nc.tensor.matmul(ph0[:], lhsT=ww0t_t[:], rhs=x0T[:],
                                 start=True, stop=False)
                nc.tensor.matmul(ph0[:], lhsT=ww0b_t[:], rhs=agg1T[:],
                                 start=False, stop=True)
                h0 = p2b.tile([64, 256], F32)
                nc.scalar.activation(h0[:], ph0[:],
                                     mybir.ActivationFunctionType.Relu, bias=bw0_t[:])

                # out = Ww1t.T@h0 + Ww1b.T@aggh1 + bw1
                po = ps2.tile([64, 256], F32, tag="ph", name="po")
                nc.tensor.matmul(po[:], lhsT=ww1t_t[:], rhs=h0[:],
                                 start=True, stop=False)
                nc.tensor.matmul(po[:], lhsT=ww1b_t[:], rhs=aggh1[:],
                                 start=False, stop=True)
                out_fm = p2b.tile([64, 256], F32)
                nc.vector.tensor_tensor(out=out_fm[:], in0=po[:],
                                        in1=bw1_t[:].to_broadcast([64, 256]),
                                        op=mybir.AluOpType.add)
                out_nm = p2b.tile([128, 2, 64], F32)
                for j in range(2):
                    tro = ps2t.tile([128, 128], F32, tag="tr2", name="tro")
                    nc.tensor.transpose(out=tro[:, 0:64],
                                        in_=out_fm[:, j * 128:(j + 1) * 128],
                                        identity=ident[:64, :64])
                    nc.vector.tensor_copy(out=out_nm[:, j, :], in_=tro[:, 0:64])
                # DRAM row r = p*2+u  (host un-permutes)
                nc.sync.dma_start(
                    out=out[:].rearrange("(p u) e -> p u e", u=2), in_=out_nm[:])

    nc.compile()
    return nc


def _prep_inputs(inputs):
    """Host-side sharding/layout. Returns in_maps (list of 8 dicts)."""
    gi = {k: np.asarray(v) for k, v in inputs.items()}

    def pad_rows(a, n):
        if a.shape[0] == n:
            return a
        pad = np.zeros((n - a.shape[0],) + a.shape[1:], a.dtype)
        return np.concatenate([a, pad], axis=0)

    # ---- weights (shared) ----
    Wpu = gi["Wproj_u"].astype(np.float32)
    Wpi = gi["Wproj_i"].astype(np.float32)
    W_user = np.concatenate([
        Wpu[0:64], Wpu[64:128] / 10.0, Wpu[128:224] / 8.0, Wpu[224:524],
        gi["Wnum_u"].astype(np.float32)], axis=0)
    b_user = (gi["bproj_u"] + gi["bnum_u"]).astype(np.float32)
    W_item = np.concatenate([
        Wpi[0:64], Wpi[64:128] / 10.0, Wpi[128:224] / 8.0, Wpi[224:1292],
        gi["Wnum_i"].astype(np.float32)], axis=0)
    b_item = (gi["bproj_i"] + gi["bnum_i"]).astype(np.float32)
    shared = {
        "W_user": np.ascontiguousarray(W_user),
        "b_user": b_user,
        "W_item": np.ascontiguousarray(W_item),
        "b_item": b_item,
        "Wv0": np.ascontiguousarray(gi["W_v"][0] / 20.0).astype(np.float32),
        "bv0": (gi["b_v"][0] / 20.0).astype(np.float32),
        "Wv1": np.ascontiguousarray(gi["W_v"][1] / 20.0).astype(np.float32),
        "bv1": (gi["b_v"][1] / 20.0).astype(np.float32),
        "Ww0t": np.ascontiguousarray(gi["W_w"][0][:64]).astype(np.float32),
        "Ww0b": np.ascontiguousarray(gi["W_w"][0][64:]).astype(np.float32),
        "bw0": gi["b_w"][0].astype(np.float32),
        "Ww1t": np.ascontiguousarray(gi["W_w"][1][:64]).astype(np.float32),
        "Ww1b": np.ascontiguousarray(gi["W_w"][1][64:]).astype(np.float32),
        "bw1": gi["b_w"][1].astype(np.float32),
        "ufe": np.ascontiguousarray(gi["user_feat_emb"]).astype(np.float32),
        "ife": np.ascontiguousarray(gi["item_feat_emb"]).astype(np.float32),
        "wordp": np.concatenate(
            [gi["word_emb"].astype(np.float32),
             np.zeros((VOCAB, 32), np.float32)], axis=1),
    }

    seeds_r = _remap_nodes(gi["seeds"])
    nb1_r = _remap_nodes(gi["neigh1"])
    nb2_r = _remap_nodes(gi["neigh2"])

    in_maps = []
    for c in range(NCORES):
        us = slice(c * U_PC, (c + 1) * U_PC)
        isl = slice(c * I_PC, (c + 1) * I_PC)
        m = dict(shared)
        m["uidT"] = np.ascontiguousarray(
            pad_rows(gi["user_id_emb"][us], U_PAD).T).astype(np.float32)
        m["uw300T"] = np.ascontiguousarray(
            pad_rows(gi["user_word300"][us], U_PAD).T).astype(np.float32)
        m["unumT"] = np.ascontiguousarray(
            pad_rows(gi["user_numeric"][us], U_PAD).T).astype(np.float32)
        m["iidT"] = np.ascontiguousarray(
            pad_rows(gi["item_id_emb"][isl], I_PAD).T).astype(np.float32)
        m["iw300T"] = np.ascontiguousarray(
            pad_rows(gi["item_word300"][isl], I_PAD).T).astype(np.float32)
        m["is768T"] = np.ascontiguousarray(
            pad_rows(gi["item_sent768"][isl], I_PAD).T).astype(np.float32)
        m["inumT"] = np.ascontiguousarray(
            pad_rows(gi["item_numeric"][isl], I_PAD).T).astype(np.float32)

        # phase-1 gather indices
        uf = pad_rows(np.asarray(gi["user_feat_idx"][us]), U_PAD)  # [U_PAD, 10]
        # flat[t, (j*10+f)*128 + p] = uf[t*512 + j*128 + p, f]
        uff = uf.reshape(NT_U, 4, 128, 10).transpose(0, 1, 3, 2).reshape(NT_U, 5120)
        m["ufidx"] = _wrap_idx(uff)
        ut = pad_rows(np.asarray(gi["user_text_idx"][us]), U_PAD)  # [U_PAD, 3, 8]
        utf = ut.reshape(NT_U, 4, 128, 24).transpose(0, 1, 3, 2).reshape(NT_U, 12288)
        m["utidx"] = _wrap_idx(utf)
        if_ = pad_rows(np.asarray(gi["item_feat_idx"][isl]), I_PAD)
        iff = if_.reshape(NT_I, 4, 128, 10).transpose(0, 1, 3, 2).reshape(NT_I, 5120)
        m["ifidx"] = _wrap_idx(iff)
        it_ = pad_rows(np.asarray(gi["item_text_idx"][isl]), I_PAD)
        itf = it_.reshape(NT_I, 4, 128, 24).transpose(0, 1, 3, 2).reshape(NT_I, 12288)
        m["itidx"] = _wrap_idx(itf)

        # phase-2 indices
        bs = slice(c * B_PC, (c + 1) * B_PC)
        nb2c = nb2_r[bs].reshape(PAIRS, 20)      # pair-major [5120, 20]
        # stage-b element order: e = (j*20+t)*128 + p <-> nb2c[c2*640+j*128+p, t]
        nb2t = nb2c.reshape(NCH2, 5, 128, 20).transpose(0, 1, 3, 2)  # [8,5,20,128]
        rows = nb2t.reshape(NCH2, 12800).astype(np.int64)
        a_idx = np.zeros((NCH2, 8, CAP2), np.int16)
        b_idx = np.zeros((NCH2, 12800), np.int64)
        for c2 in range(NCH2):
            r = rows[c2]
            o = r // NODES_PC
            loc = r % NODES_PC
            counts = np.bincount(o, minlength=8)
            if counts.max() > CAP2:
                raise ValueError(
                    f"neigh2 region bucket overflow: {counts.max()} > {CAP2}")
            starts = np.concatenate(([0], np.cumsum(counts)[:-1]))
            order = np.argsort(o, kind="stable")
            pos = np.empty(12800, np.int64)
            pos[order] = np.arange(12800) - starts[o[order]]
            b_idx[c2] = o * CAP2 + pos
            locs_sorted = loc[order]
            for oo in range(8):
                a_idx[c2, oo, :counts[oo]] = locs_sorted[
                    starts[oo]:starts[oo] + counts[oo]]
        m["nb2aidx"] = _wrap_idx(a_idx)
        m["nb2bidx"] = _wrap_idx(b_idx.astype(np.int16))
        nb1c = nb1_r[bs].reshape(PAIRS)          # [5120]
        m["x1idx"] = np.ascontiguousarray(
            nb1c.reshape(NCH2, 5, 128)).astype(np.int32)
        # seed order: [k=u*20+t, p] = nb1[u*128+p, t]
        nb1s = nb1_r[bs].reshape(2, 128, 20).transpose(0, 2, 1)  # [2, 20, 128]
        m["nb1sidx"] = np.ascontiguousarray(
            nb1s.reshape(40, 128)).astype(np.int32)
        m["seedidx"] = np.ascontiguousarray(
            seeds_r[bs].reshape(2, 128)).astype(np.int32)
        in_maps.append(m)
    return in_maps


def kernel(**inputs) -> np.ndarray:
    global LAST_RESULT, _CACHED
    if _CACHED is None:
        _CACHED = _build_program()
    nc = _CACHED
    in_maps = _prep_inputs(inputs)
    trace = bool(int(os.environ.get("KERNEL_TRACE", "0")))
    res = run_bass_kernel_spmd(nc, in_maps, core_ids=list(range(NCORES)),
                               trace=trace)
    LAST_RESULT = res
    out = np.empty((B, 64), np.float32)
    for c in range(NCORES):
        oc = res.results[c]["out"].reshape(128, 2, 64)
        out[c * B_PC:(c + 1) * B_PC] = (
            oc.transpose(1, 0, 2).reshape(B_PC, 64))
    return out


# revision 14
# speedup vs baseline: 1.4884x; 1.0382x over previous
"""Distributed Bass kernel for nn_ASAGE (GraphSAGE over a two-tower user/item
graph) on 8 TRN2 NeuronCores.

Strategy:
  Phase 1 (node embeddings, node-sharded): each core computes x = proj(feats)
  and v0 = x @ (W_v0/20) for its 1/8 slice of users+items. Embedding-bag
  lookups (feature bags, text tokens) run as big int16 `dma_gather`s from
  DRAM tables; dense features are host-pre-transposed so they DMA straight
  into feature-major matmul operands.
  AllGather of [x||v0] (one [19456,128] f32 tile per core -> [155648,128]).
  Phase 2 (data-parallel over seeds): per-core 256 seeds; v0 rows for
  neighbor aggregation gathered via indirect DMA (int32 indices); two
  GraphSAGE layers as feature-major matmuls.

Host-side work is restricted to layout transformations: slicing/sharding,
integer index remapping/reordering, weight concat + folding of the constant
bag-mean scales (1/10, 1/8, 1/20) into the corresponding weight rows.
"""
import os
import sys
import types

sys.path.insert(0, "/opt/trn_rl_repo")

import numpy as np


def _ensure_axon_hooks():
    """Provide antenv.axon_hooks + register the NTFF profile hook (the agent
    image's antenv lacks it). Needed only for trace=True timing runs."""
    try:
        import antenv
    except ImportError:
        return
    if "antenv.axon_hooks" in sys.modules:
        return
    mod = types.ModuleType("antenv.axon_hooks")
    mod._HOOK = None

    def set_axon_ntff_profile_hook(hook, _mod=mod):
        _mod._HOOK = hook

    def get_axon_ntff_profile_hook(_mod=mod):
        return _mod._HOOK

    mod.set_axon_ntff_profile_hook = set_axon_ntff_profile_hook
    mod.get_axon_ntff_profile_hook = get_axon_ntff_profile_hook
    sys.modules["antenv.axon_hooks"] = mod
    antenv.axon_hooks = mod
    try:
        from trn_agent_boot.trn_boot import _ntff_profile_via_ctypes

        mod.set_axon_ntff_profile_hook(
            _ntff_profile_via_ctypes("/opt/axon/libaxon_pjrt.so")
        )
    except Exception:
        pass


_ensure_axon_hooks()

import concourse.bass as bass
import concourse.bacc as bacc
import concourse.tile as tile
import concourse.mybir as mybir
from concourse.bass_utils import run_bass_kernel_spmd
from concourse.masks import make_identity

# ---- problem constants (hardcoded per contract) ----
N_USER = 100000
M_ITEM = 50000
D = 64
WD = 32
B = 2048
S = 20
VOCAB = 20000
NCORES = 8

U_PC = N_USER // NCORES          # 12500 real users per core
I_PC = M_ITEM // NCORES          # 6250 real items per core
U_PAD = 12800                    # 25 tiles of 512
I_PAD = 6656                     # 13 tiles of 512
NT_U = U_PAD // 512
NT_I = I_PAD // 512
NODES_PC = U_PAD + I_PAD         # 19456
XV_ROWS = NODES_PC * NCORES      # 155648
B_PC = B // NCORES               # 256
PAIRS = B_PC * S                 # 5120
NCH2 = 8                         # neigh2 chunks
PAIRS_CH = PAIRS // NCH2         # 640 = 5 * 128
CAP2 = 1792                      # per-(chunk, region) stage-a bucket capacity

F32 = mybir.dt.float32
BF16 = mybir.dt.bfloat16
I32 = mybir.dt.int32
I16 = mybir.dt.int16

LAST_RESULT = None  # BassKernelResults of the most recent run (for test.py)

_CACHED = None  # (nc, static_in_map) — compile once per process


def _remap_nodes(idx):
    """Map reference node ids (users 0..1e5, items 1e5..1.5e5) to AG row ids."""
    idx = np.asarray(idx, dtype=np.int64)
    user = idx < N_USER
    out = np.empty_like(idx)
    u = idx[user]
    out[user] = (u // U_PC) * NODES_PC + (u % U_PC)
    m = idx[~user] - N_USER
    out[~user] = (m // I_PC) * NODES_PC + U_PAD + (m % I_PC)
    return out.astype(np.int32)


def _wrap_idx(flat):
    """dma_gather int16 index layout: [128, n/16], idx for element i at
    partition i%16 (replicated to all 8 16-partition groups), free slot i//16."""
    flat = np.asarray(flat, dtype=np.int16)
    n = flat.shape[-1]
    lead = flat.shape[:-1]
    w = flat.reshape(*lead, n // 16, 16)
    w = np.swapaxes(w, -1, -2)  # [..., 16, n//16]
    return np.tile(w, (1,) * len(lead) + (8, 1)).astype(np.int16)


def _bag_sum(nc, pool, g, nsub, bag, e, name):
    """Sum over the `bag` axis of g viewed as [128, nsub, bag, e].
    Returns a contiguous [128, nsub, e] tile. bag in (8, 10, 20)."""
    v = g.rearrange("p (j b) e -> p j b e", b=bag)
    if bag == 8:
        a1 = pool.tile([128, nsub, 4, e], F32, tag=f"{name}a1", name=f"{name}a1")
        nc.vector.tensor_tensor(out=a1[:], in0=v[:, :, 0:4, :], in1=v[:, :, 4:8, :],
                                op=mybir.AluOpType.add)
        a2 = pool.tile([128, nsub, 2, e], F32, tag=f"{name}a2", name=f"{name}a2")
        nc.vector.tensor_tensor(out=a2[:], in0=a1[:, :, 0:2, :], in1=a1[:, :, 2:4, :],
                                op=mybir.AluOpType.add)
        a3 = pool.tile([128, nsub, e], F32, tag=f"{name}a3", name=f"{name}a3")
        nc.vector.tensor_tensor(out=a3[:], in0=a2[:, :, 0, :], in1=a2[:, :, 1, :],
                                op=mybir.AluOpType.add)
        return a3
    if bag == 10:
        s1 = pool.tile([128, nsub, 5, e], F32, tag=f"{name}s1", name=f"{name}s1")
        nc.vector.tensor_tensor(out=s1[:], in0=v[:, :, 0:5, :], in1=v[:, :, 5:10, :],
                                op=mybir.AluOpType.add)
        s2 = pool.tile([128, nsub, 2, e], F32, tag=f"{name}s2", name=f"{name}s2")
        nc.vector.tensor_tensor(out=s2[:], in0=s1[:, :, 0:2, :], in1=s1[:, :, 2:4, :],
                                op=mybir.AluOpType.add)
        s3 = pool.tile([128, nsub, e], F32, tag=f"{name}s3", name=f"{name}s3")
        nc.vector.tensor_tensor(out=s3[:], in0=s2[:, :, 0, :], in1=s2[:, :, 1, :],
                                op=mybir.AluOpType.add)
        s4 = pool.tile([128, nsub, e], F32, tag=f"{name}s4", name=f"{name}s4")
        nc.vector.tensor_tensor(out=s4[:], in0=s3[:], in1=s1[:, :, 4, :],
                                op=mybir.AluOpType.add)
        return s4
    assert bag == 20
    a = pool.tile([128, nsub, 10, e], F32, tag=f"{name}a", name=f"{name}a")
    nc.vector.tensor_tensor(out=a[:], in0=v[:, :, 0:10, :], in1=v[:, :, 10:20, :],
                            op=mybir.AluOpType.add)
    b5 = pool.tile([128, nsub, 5, e], F32, tag=f"{name}b", name=f"{name}b")
    nc.vector.tensor_tensor(out=b5[:], in0=a[:, :, 0:5, :], in1=a[:, :, 5:10, :],
                            op=mybir.AluOpType.add)
    c2 = pool.tile([128, nsub, 2, e], F32, tag=f"{name}c", name=f"{name}c")
    nc.vector.tensor_tensor(out=c2[:], in0=b5[:, :, 0:2, :], in1=b5[:, :, 2:4, :],
                            op=mybir.AluOpType.add)
    d = pool.tile([128, nsub, e], F32, tag=f"{name}d", name=f"{name}d")
    nc.vector.tensor_tensor(out=d[:], in0=c2[:, :, 0, :], in1=c2[:, :, 1, :],
                            op=mybir.AluOpType.add)
    ee = pool.tile([128, nsub, e], F32, tag=f"{name}e", name=f"{name}e")
    nc.vector.tensor_tensor(out=ee[:], in0=d[:], in1=b5[:, :, 4, :],
                            op=mybir.AluOpType.add)
    return ee


def _build_program():
    nc = bacc.Bacc("TRN2", target_bir_lowering=False, debug=False,
                   num_devices=NCORES, num_swdge_queues=4)
    dt = nc.dram_tensor

    # dense (host pre-transposed, padded) per-core inputs
    uidT = dt("uidT", [64, U_PAD], F32, kind="ExternalInput")
    uw300T = dt("uw300T", [300, U_PAD], F32, kind="ExternalInput")
    unumT = dt("unumT", [10, U_PAD], F32, kind="ExternalInput")
    iidT = dt("iidT", [64, I_PAD], F32, kind="ExternalInput")
    iw300T = dt("iw300T", [300, I_PAD], F32, kind="ExternalInput")
    is768T = dt("is768T", [768, I_PAD], F32, kind="ExternalInput")
    inumT = dt("inumT", [10, I_PAD], F32, kind="ExternalInput")

    # gather tables (replicated)
    ufe = dt("ufe", [30000, 64], F32, kind="ExternalInput")
    ife = dt("ife", [30000, 64], F32, kind="ExternalInput")
    wordp = dt("wordp", [VOCAB, 64], F32, kind="ExternalInput")  # padded 32->64

    # gather indices
    ufidx = dt("ufidx", [NT_U, 128, 320], I16, kind="ExternalInput")
    utidx = dt("utidx", [NT_U, 128, 768], I16, kind="ExternalInput")
    ifidx = dt("ifidx", [NT_I, 128, 320], I16, kind="ExternalInput")
    itidx = dt("itidx", [NT_I, 128, 768], I16, kind="ExternalInput")
    nb2aidx = dt("nb2aidx", [NCH2, 8, 128, CAP2 // 16], I16, kind="ExternalInput")
    nb2bidx = dt("nb2bidx", [NCH2, 128, 800], I16, kind="ExternalInput")
    x1idx = dt("x1idx", [NCH2, 5, 128], I32, kind="ExternalInput")
    nb1sidx = dt("nb1sidx", [40, 128], I32, kind="ExternalInput")
    seedidx = dt("seedidx", [2, 128], I32, kind="ExternalInput")

    # weights (replicated; host-packed, scales folded)
    W_user = dt("W_user", [534, 64], F32, kind="ExternalInput")
    b_user = dt("b_user", [64], F32, kind="ExternalInput")
    W_item = dt("W_item", [1302, 64], F32, kind="ExternalInput")
    b_item = dt("b_item", [64], F32, kind="ExternalInput")
    Wv0 = dt("Wv0", [64, 64], F32, kind="ExternalInput")     # /20 folded
    bv0 = dt("bv0", [64], F32, kind="ExternalInput")
    Wv1 = dt("Wv1", [64, 64], F32, kind="ExternalInput")     # /20 folded
    bv1 = dt("bv1", [64], F32, kind="ExternalInput")
    Ww0t = dt("Ww0t", [64, 64], F32, kind="ExternalInput")
    Ww0b = dt("Ww0b", [64, 64], F32, kind="ExternalInput")
    bw0 = dt("bw0", [64], F32, kind="ExternalInput")
    Ww1t = dt("Ww1t", [64, 64], F32, kind="ExternalInput")
    Ww1b = dt("Ww1b", [64, 64], F32, kind="ExternalInput")
    bw1 = dt("bw1", [64], F32, kind="ExternalInput")

    out = dt("out", [B_PC, 64], F32, kind="ExternalOutput")

    # user/item segment row-ranges in W_user/W_item
    useg = [("id", 0, 64), ("feat", 64, 128), ("text", 128, 224),
            ("w3a", 224, 352), ("w3b", 352, 480), ("w3c", 480, 524),
            ("num", 524, 534)]
    iseg = [("id", 0, 64), ("feat", 64, 128), ("text", 128, 224),
            ("w3a", 224, 352), ("w3b", 352, 480), ("w3c", 480, 524),
            ("s7a", 524, 652), ("s7b", 652, 780), ("s7c", 780, 908),
            ("s7d", 908, 1036), ("s7e", 1036, 1164), ("s7f", 1164, 1292),
            ("num", 1292, 1302)]

    qctr = [0]

    def next_q():
        q = qctr[0] % 4
        qctr[0] += 1
        return q

    with tile.TileContext(nc) as tc:
        with tc.tile_pool(name="const", bufs=1) as constp, \
             tc.tile_pool(name="dram", bufs=1, space="DRAM") as dram:
            ident = constp.tile([128, 128], F32)
            make_identity(nc, ident[:])

            def load_w(t, rows=64):
                w = constp.tile([rows, 64], F32, name=f"w_{t.name}", uniquify=True)
                nc.sync.dma_start(out=w[:], in_=t[:])
                return w

            def load_b(t):
                b_ = constp.tile([64, 1], F32, name=f"b_{t.name}", uniquify=True)
                nc.sync.dma_start(out=b_[:], in_=t[:, None])
                return b_

            wu = {}
            for nm, r0, r1 in useg:
                w = constp.tile([r1 - r0, 64], F32, name=f"wu_{nm}")
                nc.sync.dma_start(out=w[:], in_=W_user[r0:r1, :])
                wu[nm] = w
            wi = {}
            for nm, r0, r1 in iseg:
                w = constp.tile([r1 - r0, 64], F32, name=f"wi_{nm}")
                nc.sync.dma_start(out=w[:], in_=W_item[r0:r1, :])
                wi[nm] = w
            bu_t = load_b(b_user)
            bi_t = load_b(b_item)
            wv0_t = load_w(Wv0)
            bv0_t = load_b(bv0)
            wv1_t = load_w(Wv1)
            bv1_t = load_b(bv1)
            ww0t_t = load_w(Ww0t)
            ww0b_t = load_w(Ww0b)
            bw0_t = load_b(bw0)
            ww1t_t = load_w(Ww1t)
            ww1b_t = load_w(Ww1b)
            bw1_t = load_b(bw1)

            xv_slice = dram.tile([NODES_PC, 128], BF16)
            xv_full = dram.tile([XV_ROWS, 128], BF16, addr_space="Shared")
            ident_bf = constp.tile([128, 128], BF16)
            nc.vector.tensor_copy(out=ident_bf[:], in_=ident[:])

            # ---------------- phase 1 ----------------
            with tc.tile_pool(name="p1", bufs=2) as p1, \
                 tc.tile_pool(name="p1r", bufs=2) as p1r, \
                 tc.tile_pool(name="ps_acc", bufs=2, space="PSUM") as ps_acc, \
                 tc.tile_pool(name="ps_tr", bufs=2, space="PSUM") as ps_tr, \
                 tc.tile_pool(name="ps_v0", bufs=1, space="PSUM") as ps_v0:

                def node_tile(kind, t):
                    if kind == "u":
                        segs, wseg, bseg = useg, wu, bu_t
                        fidx, tidx, ftab = ufidx, utidx, ufe
                        idT, w300T, numT, s768T = uidT, uw300T, unumT, None
                        row0 = t * 512
                    else:
                        segs, wseg, bseg = iseg, wi, bi_t
                        fidx, tidx, ftab = ifidx, itidx, ife
                        idT, w300T, numT, s768T = iidT, iw300T, inumT, is768T
                        row0 = U_PAD + t * 512
                    cols = slice(t * 512, (t + 1) * 512)

                    # ---- gathers (split for 4-queue concurrency) ----
                    fit = p1.tile([128, 320], I16, tag="fit", name="fit")
                    nc.sync.dma_start(out=fit[:], in_=fidx[t])
                    gf = p1.tile([128, 40, 64], F32, tag="gf", name="gf")
                    nc.gpsimd.dma_gather(out_ap=gf[:], in_ap=ftab[:], idxs_ap=fit[:],
                                         num_idxs=5120, num_idxs_reg=5120,
                                         elem_size=64, single_packet=False,
                                         queue_num=next_q())
                    tit = p1.tile([128, 768], I16, tag="tit", name="tit")
                    nc.sync.dma_start(out=tit[:], in_=tidx[t])
                    gt = p1.tile([128, 96, 64], F32, tag="gt", name="gt")
                    # three 4096-idx sub-gathers into thirds of gt; idx sub-slab
                    # s covers elements [s*4096, (s+1)*4096) = blocks 32s..32s+32
                    for sgi in range(3):
                        nc.gpsimd.dma_gather(
                            out_ap=gt[:, sgi * 32:(sgi + 1) * 32, :],
                            in_ap=wordp[:],
                            idxs_ap=tit[:, sgi * 256:(sgi + 1) * 256],
                            num_idxs=4096, num_idxs_reg=4096,
                            elem_size=64, single_packet=False,
                            queue_num=next_q())

                    # ---- bag sums (feat: 10 -> [128,4,64]; text: 8 -> [128,12,64])
                    feat_nm = _bag_sum(nc, p1, gf[:], 4, 10, 64, "f")
                    text_nm = _bag_sum(nc, p1, gt[:], 12, 8, 64, "t")
                    # text: keep only first 32 cols, contiguous [128, 4, 3, 32]
                    text_c = p1.tile([128, 4, 3, 32], F32, tag="textc", name="textc")
                    nc.vector.tensor_copy(
                        out=text_c[:],
                        in_=text_nm[:].rearrange("p (j s) e -> p j s e", s=3)[:, :, :, 0:32])

                    # ---- transposes into feature-major rhs ----
                    rhs_feat = p1r.tile([64, 512], F32, tag="rhs_feat", name="rhs_feat")
                    rhs_text = p1r.tile([96, 512], F32, tag="rhs_text", name="rhs_text")
                    for j in range(4):
                        trf = ps_tr.tile([128, 128], F32, tag="tr", name="trf")
                        nc.tensor.transpose(out=trf[:64, :], in_=feat_nm[:, j, :],
                                            identity=ident[:])
                        nc.scalar.copy(out=rhs_feat[:, j * 128:(j + 1) * 128],
                                       in_=trf[:64, :])
                        trt = ps_tr.tile([128, 128], F32, tag="tr", name="trt")
                        nc.tensor.transpose(out=trt[:96, :], in_=text_c[:, j],
                                            identity=ident[:])
                        nc.vector.tensor_copy(out=rhs_text[:, j * 128:(j + 1) * 128],
                                              in_=trt[:96, :])

                    # ---- dense rhs tiles ----
                    rhs = {"feat": rhs_feat, "text": rhs_text}
                    rhs_id = p1r.tile([64, 512], F32, tag="rhs_id", name="rhs_id")
                    nc.sync.dma_start(out=rhs_id[:], in_=idT[:, cols])
                    rhs["id"] = rhs_id
                    for k, nm in enumerate(["w3a", "w3b", "w3c"]):
                        rr = (128, 128, 44)[k]
                        rt = p1r.tile([rr, 512], F32, tag=f"rhs_{nm}", name=f"rhs_{nm}")
                        nc.sync.dma_start(out=rt[:],
                                          in_=w300T[k * 128:k * 128 + rr, cols])
                        rhs[nm] = rt
                    rhs_num = p1r.tile([10, 512], F32, tag="rhs_num", name="rhs_num")
                    nc.sync.dma_start(out=rhs_num[:], in_=numT[:, cols])
                    rhs["num"] = rhs_num
                    if s768T is not None:
                        for k in range(6):
                            nm = f"s7{'abcdef'[k]}"
                            rt = p1r.tile([128, 512], F32, tag=f"rhs_{nm}",
                                          name=f"rhs_{nm}")
                            nc.sync.dma_start(out=rt[:],
                                              in_=s768T[k * 128:(k + 1) * 128, cols])
                            rhs[nm] = rt

                    # ---- projection matmul (accumulate over segments) ----
                    acc = ps_acc.tile([64, 512], F32, tag="acc", name="acc")
                    for k, (nm, r0, r1) in enumerate(segs):
                        nc.tensor.matmul(acc[:], lhsT=wseg[nm][:], rhs=rhs[nm][:],
                                         start=(k == 0), stop=(k == len(segs) - 1))
                    x_fm = p1.tile([64, 512], F32, tag="x_fm", name="x_fm")
                    nc.vector.tensor_tensor(out=x_fm[:], in0=acc[:],
                                            in1=bseg[:].to_broadcast([64, 512]),
                                            op=mybir.AluOpType.add)
                    pv0 = ps_v0.tile([64, 512], F32, tag="pv0", name="pv0")
                    nc.tensor.matmul(pv0[:], lhsT=wv0_t[:], rhs=x_fm[:],
                                     start=True, stop=True)
                    v0_fm = p1.tile([64, 512], F32, tag="v0_fm", name="v0_fm")
                    nc.vector.tensor_tensor(out=v0_fm[:], in0=pv0[:],
                                            in1=bv0_t[:].to_broadcast([64, 512]),
                                            op=mybir.AluOpType.add)

                    # ---- transpose back to node-major [128,128] and store ----
                    for j in range(4):
                        jc = slice(j * 128, (j + 1) * 128)
                        xv_sb = p1.tile([128, 128], BF16, tag="xv_sb", name="xv_sb")
                        trx = ps_tr.tile([128, 128], F32, tag="tr", name="trx")
                        nc.tensor.transpose(out=trx[:, 0:64], in_=x_fm[:, jc],
                                            identity=ident[:64, :64])
                        nc.tensor.transpose(out=trx[:, 64:128], in_=v0_fm[:, jc],
                                            identity=ident[:64, :64])
                        nc.vector.tensor_copy(out=xv_sb[:], in_=trx[:])
                        nc.sync.dma_start(
                            out=xv_slice[row0 + j * 128: row0 + (j + 1) * 128, :],
                            in_=xv_sb[:])

                for t in range(NT_U):
                    node_tile("u", t)
                for t in range(NT_I):
                    node_tile("i", t)

            # ---------------- all-gather ----------------
            nc.gpsimd.collective_compute(
                "AllGather", mybir.AluOpType.bypass,
                replica_groups=[list(range(NCORES))],
                ins=[xv_slice.opt()], outs=[xv_full.opt()])

            # ---------------- phase 2 ----------------
            with tc.tile_pool(name="p2", bufs=2) as p2, \
                 tc.tile_pool(name="p2b", bufs=1) as p2b, \
                 tc.tile_pool(name="ps2", bufs=2, space="PSUM") as ps2, \
                 tc.tile_pool(name="ps2t", bufs=2, space="PSUM") as ps2t:

                aggh1 = p2b.tile([64, 256], F32)

                seqs = [dram.tile([8 * CAP2, 128], BF16, name=f"seq{i}")
                        for i in range(NCH2)]
                for c2 in range(NCH2):
                    # stage-a: region-bucketed gathers xv_full -> seq (v0 half)
                    for o in range(8):
                        ia = p2.tile([128, CAP2 // 16], I16, tag="ia", name="ia")
                        nc.sync.dma_start(out=ia[:], in_=nb2aidx[c2, o])
                        ga = p2.tile([128, CAP2 // 128, 128], BF16, tag="ga",
                                     name="ga")
                        nc.gpsimd.dma_gather(
                            out_ap=ga[:],
                            in_ap=xv_full[o * NODES_PC:(o + 1) * NODES_PC, :],
                            idxs_ap=ia[:], num_idxs=CAP2, num_idxs_reg=CAP2,
                            elem_size=128, single_packet=False,
                            queue_num=next_q())
                        nc.sync.dma_start(
                            out=seqs[c2][o * CAP2:(o + 1) * CAP2, :].rearrange(
                                "(b p) e -> p b e", p=128),
                            in_=ga[:])
                    # stage-b: permute into reduce layout
                    ib = p2.tile([128, 800], I16, tag="ib", name="ib")
                    nc.sync.dma_start(out=ib[:], in_=nb2bidx[c2])
                    g2 = p2.tile([128, 100, 128], BF16, tag="g2", name="g2")
                    for sgi in range(4):
                        nc.gpsimd.dma_gather(
                            out_ap=g2[:, sgi * 25:(sgi + 1) * 25, :],
                            in_ap=seqs[c2][:],
                            idxs_ap=ib[:, sgi * 200:(sgi + 1) * 200],
                            num_idxs=3200, num_idxs_reg=3200,
                            elem_size=128, single_packet=False,
                            queue_num=next_q())
                    agg2_nm = _bag_sum(nc, p2, g2[:, :, 64:128], 5, 20, 64, "g2")

                    itx1 = p2.tile([128, 5], I32, tag="itx1", name="itx1")
                    nc.sync.dma_start(out=itx1[:],
                                      in_=x1idx[c2].rearrange("k p -> p k"))
                    gx1 = p2.tile([128, 5, 64], BF16, tag="gx1", name="gx1")
                    for k in range(5):
                        nc.gpsimd.indirect_dma_start(
                            out=gx1[:, k, :], out_offset=None, in_=xv_full[:],
                            in_offset=bass.IndirectOffsetOnAxis(
                                ap=itx1[:, k:k + 1], axis=0))

                    x1T = p2.tile([64, 640], F32, tag="x1T", name="x1T")
                    agg2T = p2.tile([64, 640], F32, tag="agg2T", name="agg2T")
                    for j in range(5):
                        col = slice(j * 128, (j + 1) * 128)
                        tra = ps2t.tile([128, 128], F32, tag="tr2", name="tra")
                        nc.tensor.transpose(out=tra[:64, :], in_=agg2_nm[:, j, :],
                                            identity=ident[:])
                        nc.vector.tensor_copy(out=agg2T[:, col], in_=tra[:64, :])
                        trx1 = ps2t.tile([128, 128], BF16, tag="tr2b", name="trx1")
                        nc.tensor.transpose(out=trx1[:64, :], in_=gx1[:, j, :],
                                            identity=ident_bf[:])
                        nc.vector.tensor_copy(out=x1T[:, col], in_=trx1[:64, :])

                    # h1 = relu(Ww0t.T@x1T + Ww0b.T@agg2T + bw0); v1 = Wv1.T@h1+bv1
                    # per sub-chunk of 320 pairs (= 16 seeds)
                    v1c = p2.tile([64, 640], F32, tag="v1c", name="v1c")
                    for n in range(2):
                        col = slice(n * 320, (n + 1) * 320)
                        ph = ps2.tile([64, 320], F32, tag="ph", name="ph")
                        nc.tensor.matmul(ph[:], lhsT=ww0t_t[:], rhs=x1T[:, col],
                                         start=True, stop=False)
                        nc.tensor.matmul(ph[:], lhsT=ww0b_t[:], rhs=agg2T[:, col],
                                         start=False, stop=True)
                        h1c = p2.tile([64, 320], F32, tag="h1c", name="h1c")
                        nc.scalar.activation(h1c[:], ph[:],
                                             mybir.ActivationFunctionType.Relu,
                                             bias=bw0_t[:])
                        pv = ps2.tile([64, 320], F32, tag="pv", name="pv")
                        nc.tensor.matmul(pv[:], lhsT=wv1_t[:], rhs=h1c[:],
                                         start=True, stop=True)
                        nc.vector.tensor_tensor(
                            out=v1c[:, col], in0=pv[:],
                            in1=bv1_t[:].to_broadcast([64, 320]),
                            op=mybir.AluOpType.add)
                    # agg_h1 for these 32 seeds: sum over s of [64, 32, 20]
                    vv = v1c[:].rearrange("p (b s) -> p b s", s=20)
                    va = p2.tile([64, 32, 10], F32, tag="va", name="va")
                    nc.vector.tensor_tensor(out=va[:], in0=vv[:, :, 0:10],
                                            in1=vv[:, :, 10:20],
                                            op=mybir.AluOpType.add)
                    vb = p2.tile([64, 32, 5], F32, tag="vb", name="vb")
                    nc.vector.tensor_tensor(out=vb[:], in0=va[:, :, 0:5],
                                            in1=va[:, :, 5:10],
                                            op=mybir.AluOpType.add)
                    vc = p2.tile([64, 32, 2], F32, tag="vc", name="vc")
                    nc.vector.tensor_tensor(out=vc[:], in0=vb[:, :, 0:2],
                                            in1=vb[:, :, 2:4],
                                            op=mybir.AluOpType.add)
                    vd = p2.tile([64, 32], F32, tag="vd", name="vd")
                    nc.vector.tensor_tensor(out=vd[:], in0=vc[:, :, 0],
                                            in1=vc[:, :, 1],
                                            op=mybir.AluOpType.add)
                    nc.vector.tensor_tensor(
                        out=aggh1[:, c2 * 32:(c2 + 1) * 32], in0=vd[:],
                        in1=vb[:, :, 4], op=mybir.AluOpType.add)

                # neigh1 in seed order -> agg1
                itn1 = p2b.tile([128, 40], I32)
                nc.sync.dma_start(out=itn1[:], in_=nb1sidx[:].rearrange("k p -> p k"))
                gn1 = p2b.tile([128, 40, 64], BF16)
                for k in range(40):
                    nc.gpsimd.indirect_dma_start(
                        out=gn1[:, k, :], out_offset=None, in_=xv_full[:],
                        in_offset=bass.IndirectOffsetOnAxis(
                            ap=itn1[:, k:k + 1], axis=0),
                        element_offset=64)
                agg1_nm = _bag_sum(nc, p2, gn1[:], 2, 20, 64, "n1")
                agg1T = p2b.tile([64, 256], F32)
                # seeds
                its = p2b.tile([128, 2], I32)
                nc.sync.dma_start(out=its[:], in_=seedidx[:].rearrange("k p -> p k"))
                gs = p2b.tile([128, 2, 64], BF16)
                for k in range(2):
                    nc.gpsimd.indirect_dma_start(
                        out=gs[:, k, :], out_offset=None, in_=xv_full[:],
                        in_offset=bass.IndirectOffsetOnAxis(
                            ap=its[:, k:k + 1], axis=0))
                x0T = p2b.tile([64, 256], F32)
                for j in range(2):
                    col = slice(j * 128, (j + 1) * 128)
                    tr1 = ps2t.tile([128, 128], F32, tag="tr2", name="tr1")
                    nc.tensor.transpose(out=tr1[:64, :], in_=agg1_nm[:, j, :],
                                        identity=ident[:])
                    nc.vector.tensor_copy(out=agg1T[:, col], in_=tr1[:64, :])
                    tr0 = ps2t.tile([128, 128], BF16, tag="tr2b", name="tr0")
                    nc.tensor.transpose(out=tr0[:64, :], in_=gs[:, j, :],
                                        identity=ident_bf[:])
                    nc.vector.tensor_copy(out=x0T[:, col], in_=tr0[:64, :])

                # h0 = relu(Ww0t.T@x0T + Ww0b.T@agg1T + bw0)
                ph0 = ps2.tile([64, 256], F32, tag="ph", name="ph0")
                nc.tensor.matmul(ph0[:], lhsT=ww0t_t[:], rhs=x0T[:],
                                 start=True, stop=False)
                nc.tensor.matmul(ph0[:], lhsT=ww0b_t[:], rhs=agg1T[:],
                                 start=False, stop=True)
                h0 = p2b.tile([64, 256], F32)
                nc.scalar.activation(h0[:], ph0[:],
                                     mybir.ActivationFunctionType.Relu, bias=bw0_t[:])

                # out = Ww1t.T@h0 + Ww1b.T@aggh1 + bw1
                po = ps2.tile([64, 256], F32, tag="ph", name="po")
                nc.tensor.matmul(po[:], lhsT=ww1t_t[:], rhs=h0[:],
                                 start=True, stop=False)
                nc.tensor.matmul(po[:], lhsT=ww1b_t[:], rhs=aggh1[:],
                                 start=False, stop=True)
                out_fm = p2b.tile([64, 256], F32)
                nc.vector.tensor_tensor(out=out_fm[:], in0=po[:],
                                        in1=bw1_t[:].to_broadcast([64, 256]),
                                        op=mybir.AluOpType.add)
                out_nm = p2b.tile([128, 2, 64], F32)
                for j in range(2):
                    tro = ps2t.tile([128, 128], F32, tag="tr2", name="tro")
                    nc.tensor.transpose(out=tro[:, 0:64],
                                        in_=out_fm[:, j * 128:(j + 1) * 128],
                                        identity=ident[:64, :64])
                    nc.vector.tensor_copy(out=out_nm[:, j, :], in_=tro[:, 0:64])
                # DRAM row r = p*2+u  (host un-permutes)
                nc.sync.dma_start(
                    out=out[:].rearrange("(p u) e -> p u e", u=2), in_=out_nm[:])

    nc.compile()
    return nc


def _prep_inputs(inputs):
    """Host-side sharding/layout. Returns in_maps (list of 8 dicts)."""
    gi = {k: np.asarray(v) for k, v in inputs.items()}

    def pad_rows(a, n):
        if a.shape[0] == n:
            return a
        pad = np.zeros((n - a.shape[0],) + a.shape[1:], a.dtype)
        return np.concatenate([a, pad], axis=0)

    # ---- weights (shared) ----
    Wpu = gi["Wproj_u"].astype(np.float32)
    Wpi = gi["Wproj_i"].astype(np.float32)
    W_user = np.concatenate([
        Wpu[0:64], Wpu[64:128] / 10.0, Wpu[128:224] / 8.0, Wpu[224:524],
        gi["Wnum_u"].astype(np.float32)], axis=0)
    b_user = (gi["bproj_u"] + gi["bnum_u"]).astype(np.float32)
    W_item = np.concatenate([
        Wpi[0:64], Wpi[64:128] / 10.0, Wpi[128:224] / 8.0, Wpi[224:1292],
        gi["Wnum_i"].astype(np.float32)], axis=0)
    b_item = (gi["bproj_i"] + gi["bnum_i"]).astype(np.float32)
    shared = {
        "W_user": np.ascontiguousarray(W_user),
        "b_user": b_user,
        "W_item": np.ascontiguousarray(W_item),
        "b_item": b_item,
        "Wv0": np.ascontiguousarray(gi["W_v"][0] / 20.0).astype(np.float32),
        "bv0": (gi["b_v"][0] / 20.0).astype(np.float32),
        "Wv1": np.ascontiguousarray(gi["W_v"][1] / 20.0).astype(np.float32),
        "bv1": (gi["b_v"][1] / 20.0).astype(np.float32),
        "Ww0t": np.ascontiguousarray(gi["W_w"][0][:64]).astype(np.float32),
        "Ww0b": np.ascontiguousarray(gi["W_w"][0][64:]).astype(np.float32),
        "bw0": gi["b_w"][0].astype(np.float32),
        "Ww1t": np.ascontiguousarray(gi["W_w"][1][:64]).astype(np.float32),
        "Ww1b": np.ascontiguousarray(gi["W_w"][1][64:]).astype(np.float32),
        "bw1": gi["b_w"][1].astype(np.float32),
        "ufe": np.ascontiguousarray(gi["user_feat_emb"]).astype(np.float32),
        "ife": np.ascontiguousarray(gi["item_feat_emb"]).astype(np.float32),
        "wordp": np.concatenate(
            [gi["word_emb"].astype(np.float32),
             np.zeros((VOCAB, 32), np.float32)], axis=1),
    }

    seeds_r = _remap_nodes(gi["seeds"])
    nb1_r = _remap_nodes(gi["neigh1"])
    nb2_r = _remap_nodes(gi["neigh2"])

    in_maps = []
    for c in range(NCORES):
        us = slice(c * U_PC, (c + 1) * U_PC)
        isl = slice(c * I_PC, (c + 1) * I_PC)
        m = dict(shared)
        m["uidT"] = np.ascontiguousarray(
            pad_rows(gi["user_id_emb"][us], U_PAD).T).astype(np.float32)
        m["uw300T"] = np.ascontiguousarray(
            pad_rows(gi["user_word300"][us], U_PAD).T).astype(np.float32)
        m["unumT"] = np.ascontiguousarray(
            pad_rows(gi["user_numeric"][us], U_PAD).T).astype(np.float32)
        m["iidT"] = np.ascontiguousarray(
            pad_rows(gi["item_id_emb"][isl], I_PAD).T).astype(np.float32)
        m["iw300T"] = np.ascontiguousarray(
            pad_rows(gi["item_word300"][isl], I_PAD).T).astype(np.float32)
        m["is768T"] = np.ascontiguousarray(
            pad_rows(gi["item_sent768"][isl], I_PAD).T).astype(np.float32)
        m["inumT"] = np.ascontiguousarray(
            pad_rows(gi["item_numeric"][isl], I_PAD).T).astype(np.float32)

        # phase-1 gather indices
        uf = pad_rows(np.asarray(gi["user_feat_idx"][us]), U_PAD)  # [U_PAD, 10]
        # flat[t, (j*10+f)*128 + p] = uf[t*512 + j*128 + p, f]
        uff = uf.reshape(NT_U, 4, 128, 10).transpose(0, 1, 3, 2).reshape(NT_U, 5120)
        m["ufidx"] = _wrap_idx(uff)
        ut = pad_rows(np.asarray(gi["user_text_idx"][us]), U_PAD)  # [U_PAD, 3, 8]
        utf = ut.reshape(NT_U, 4, 128, 24).transpose(0, 1, 3, 2).reshape(NT_U, 12288)
        m["utidx"] = _wrap_idx(utf)
        if_ = pad_rows(np.asarray(gi["item_feat_idx"][isl]), I_PAD)
        iff = if_.reshape(NT_I, 4, 128, 10).transpose(0, 1, 3, 2).reshape(NT_I, 5120)
        m["ifidx"] = _wrap_idx(iff)
        it_ = pad_rows(np.asarray(gi["item_text_idx"][isl]), I_PAD)
        itf = it_.reshape(NT_I, 4, 128, 24).transpose(0, 1, 3, 2).reshape(NT_I, 12288)
        m["itidx"] = _wrap_idx(itf)

        # phase-2 indices
        bs = slice(c * B_PC, (c + 1) * B_PC)
        nb2c = nb2_r[bs].reshape(PAIRS, 20)      # pair-major [5120, 20]
        # stage-b element order: e = (j*20+t)*128 + p <-> nb2c[c2*640+j*128+p, t]
        nb2t = nb2c.reshape(NCH2, 5, 128, 20).transpose(0, 1, 3, 2)  # [8,5,20,128]
        rows = nb2t.reshape(NCH2, 12800).astype(np.int64)
        a_idx = np.zeros((NCH2, 8, CAP2), np.int16)
        b_idx = np.zeros((NCH2, 12800), np.int64)
        for c2 in range(NCH2):
            r = rows[c2]
            o = r // NODES_PC
            loc = r % NODES_PC
            counts = np.bincount(o, minlength=8)
            if counts.max() > CAP2:
                raise ValueError(
                    f"neigh2 region bucket overflow: {counts.max()} > {CAP2}")
            starts = np.concatenate(([0], np.cumsum(counts)[:-1]))
            order = np.argsort(o, kind="stable")
            pos = np.empty(12800, np.int64)
            pos[order] = np.arange(12800) - starts[o[order]]
            b_idx[c2] = o * CAP2 + pos
            locs_sorted = loc[order]
            for oo in range(8):
                a_idx[c2, oo, :counts[oo]] = locs_sorted[
                    starts[oo]:starts[oo] + counts[oo]]
        m["nb2aidx"] = _wrap_idx(a_idx)
        m["nb2bidx"] = _wrap_idx(b_idx.astype(np.int16))
        nb1c = nb1_r[bs].reshape(PAIRS)          # [5120]
        m["x1idx"] = np.ascontiguousarray(
            nb1c.reshape(NCH2, 5, 128)).astype(np.int32)
        # seed order: [k=u*20+t, p] = nb1[u*128+p, t]
        nb1s = nb1_r[bs].reshape(2, 128, 20).transpose(0, 2, 1)  # [2, 20, 128]
        m["nb1sidx"] = np.ascontiguousarray(
            nb1s.reshape(40, 128)).astype(np.int32)
        m["seedidx"] = np.ascontiguousarray(
            seeds_r[bs].reshape(2, 128)).astype(np.int32)
        in_maps.append(m)
    return in_maps


def kernel(**inputs) -> np.ndarray:
    global LAST_RESULT, _CACHED
    if _CACHED is None:
        _CACHED = _build_program()
    nc = _CACHED
    in_maps = _prep_inputs(inputs)
    trace = bool(int(os.environ.get("KERNEL_TRACE", "0")))
    res = run_bass_kernel_spmd(nc, in_maps, core_ids=list(range(NCORES)),
                               trace=trace)
    LAST_RESULT = res
    out = np.empty((B, 64), np.float32)
    for c in range(NCORES):
        oc = res.results[c]["out"].reshape(128, 2, 64)
        out[c * B_PC:(c + 1) * B_PC] = (
            oc.transpose(1, 0, 2).reshape(B_PC, 64))
    return out
